# revision 1
# baseline (speedup 1.0000x reference)
"""Trainium2 Bass kernel for NonlinearElectronicEmbedding (segment softmax attention).

Strategy (data-parallel over atoms, molecule-aligned):
  - 512 molecules split as 64 consecutive molecules per core (8 cores).
  - Each molecule padded to KM tiles of 128 atoms -> every core runs the
    IDENTICAL program; per-core data differs only in tensor contents.
  - Device prelude computes k/v tables ([64,256]) from E via the ResidualMLPs
    in transposed layout (features on partitions), with the k-table fused with
    Wq:  dot(n) = x(n) . (k_mol @ Wq)[seg(n)].
  - Pass 1 per 128-atom tile: fused multiply+reduce (DVE tensor_tensor_reduce)
    against the molecule's kq row (GpSimd partition_broadcast), exp on ACT,
    per-molecule segment sum on PE (matmul accumulate into PSUM).
  - Softmax shift invariance: exp(dot/16) used directly (args bounded), no
    segment max pass needed.
  - Pass 2: out = (a / (anorm+eps)) * v_mol[seg] via ACT scaled-copy.
HBM traffic/core = x_pad in + out_pad out (~117 MB) -> memory-roofline bound.
"""

import numpy as np

F = 256
B = 512
NCORES = 8
P = 128
BC = B // NCORES  # molecules per core
G = 8  # tiles per DMA group
SQRT_F = 16.0
BETA = 1.702
EPS = 1e-8


def _build_program(KM):
    import concourse.bacc as bacc
    import concourse.mybir as mybir
    import concourse.tile as tile

    dt = mybir.dt
    f32 = dt.float32
    AF = mybir.ActivationFunctionType
    ALU = mybir.AluOpType

    TU = BC * KM  # tiles per core
    NBLK = (BC + 15) // 16

    nc = bacc.Bacc(trn_type="TRN2")

    x_h = nc.dram_tensor("x", [TU * P, F], f32, kind="ExternalInput")
    pm_h = nc.dram_tensor("pm", [P, TU], f32, kind="ExternalInput")
    ev_h = nc.dram_tensor("ev", [1, BC], f32, kind="ExternalInput")
    wkf_h = nc.dram_tensor("wkf", [1, F], f32, kind="ExternalInput")
    wvf_h = nc.dram_tensor("wvf", [1, F], f32, kind="ExternalInput")
    kw1_h = nc.dram_tensor("kw1", [P, 2, 2, P], f32, kind="ExternalInput")
    kw2_h = nc.dram_tensor("kw2", [P, 2, 2, P], f32, kind="ExternalInput")
    vw1_h = nc.dram_tensor("vw1", [P, 2, 2, P], f32, kind="ExternalInput")
    vw2_h = nc.dram_tensor("vw2", [P, 2, 2, P], f32, kind="ExternalInput")
    woqk_h = nc.dram_tensor("woqk", [P, 2, F], f32, kind="ExternalInput")
    wovv_h = nc.dram_tensor("wovv", [P, 2, F], f32, kind="ExternalInput")
    bq_h = nc.dram_tensor("bq", [1, F], f32, kind="ExternalInput")
    # biases: [P, 2] chunked; *_s pre-multiplied by BETA, *_u raw
    bkfs_h = nc.dram_tensor("bkfs", [P, 2], f32, kind="ExternalInput")
    bkfu_h = nc.dram_tensor("bkfu", [P, 2], f32, kind="ExternalInput")
    kb1s_h = nc.dram_tensor("kb1s", [P, 2], f32, kind="ExternalInput")
    kb1u_h = nc.dram_tensor("kb1u", [P, 2], f32, kind="ExternalInput")
    kb2u_h = nc.dram_tensor("kb2u", [P, 2], f32, kind="ExternalInput")
    out_h = nc.dram_tensor("out", [TU * P, F], f32, kind="ExternalOutput")
    kqtab_d = nc.dram_tensor("kqtab_scratch", [BC, F], f32)
    vtab_d = nc.dram_tensor("vtab_scratch", [BC, F], f32)

    # DRAM layout groups G tiles with partition-major rows so every DMA
    # descriptor moves G*1KB contiguous per partition (8 KB packets)
    xv = x_h[:].rearrange("(g p j) f -> p g j f", p=P, j=G)
    ov = out_h[:].rearrange("(g p j) f -> p g j f", p=P, j=G)

    with tile.TileContext(nc) as tc:
        with (
            tc.tile_pool(name="singles", bufs=1) as sg,
            tc.tile_pool(name="xpool", bufs=3) as xp,
            tc.tile_pool(name="opool", bufs=3) as op,
            tc.tile_pool(name="scr", bufs=2) as scp,
            tc.tile_pool(name="dotp", bufs=4) as dp,
        ):
            # ---- load constants ----
            def load(name, h, shape):
                t_ = sg.tile(shape, f32, tag=name)
                nc.sync.dma_start(out=t_[:], in_=h[:])
                return t_

            pm_sb = load("pm", pm_h, [P, TU])
            ev_sb = load("ev", ev_h, [1, BC])
            wkf_sb = load("wkf", wkf_h, [1, F])
            wvf_sb = load("wvf", wvf_h, [1, F])
            kw1_sb = load("kw1", kw1_h, [P, 2, 2, P])
            kw2_sb = load("kw2", kw2_h, [P, 2, 2, P])
            vw1_sb = load("vw1", vw1_h, [P, 2, 2, P])
            vw2_sb = load("vw2", vw2_h, [P, 2, 2, P])
            woqk_sb = load("woqk", woqk_h, [P, 2, F])
            wovv_sb = load("wovv", wovv_h, [P, 2, F])
            bq_sb = load("bq", bq_h, [1, F])
            bkfs_sb = load("bkfs", bkfs_h, [P, 2])
            bkfu_sb = load("bkfu", bkfu_h, [P, 2])
            kb1s_sb = load("kb1s", kb1s_h, [P, 2])
            kb1u_sb = load("kb1u", kb1u_h, [P, 2])
            kb2u_sb = load("kb2u", kb2u_h, [P, 2])

            ones1 = sg.tile([1, BC], f32)
            nc.vector.memset(ones1[:], 1.0)
            # ones rows at partitions 0/32/64/96 (PE needs lhsT and rhs on
            # the same base partition for the broadcast matmuls)
            onesp = sg.tile([P, P], f32)
            nc.vector.memset(onesp[:], 1.0)

            a_all = sg.tile([P, TU], f32)
            ahat_all = sg.tile([P, TU], f32)
            r_bc = sg.tile([P, BC], f32)
            # molecule b's row lives at partition 64*(b//32), cols (b%32)*F
            # (PE operand base partition must be 0, 32, or 64)
            kq_flat = sg.tile([P, 32 * F], f32)
            v_flat = sg.tile([P, 32 * F], f32)

            # ---- prelude: ResidualMLP in transposed layout ----
            # swish(y) = y * sigmoid(BETA*y); h_psum holds y - b.
            def swishT(c, h_psum, bs_ap, bu_ap, pre, keep_hb=False):
                sig = pre.tile([P, BC], f32, tag=f"sig_{c}")
                nc.scalar.activation(sig[:], h_psum[:], AF.Sigmoid,
                                     bias=bs_ap if bs_ap is not None else 0.0,
                                     scale=BETA)
                if bu_ap is not None:
                    hb = pre.tile([P, BC], f32, tag=f"hb_{c}")
                    nc.vector.tensor_scalar_add(hb[:], h_psum[:], bu_ap)
                elif keep_hb:
                    # residual add later needs an SBUF operand (DVE may read
                    # at most one PSUM input)
                    hb = pre.tile([P, BC], f32, tag=f"hb_{c}")
                    nc.vector.tensor_copy(hb[:], h_psum[:])
                else:
                    hb = h_psum
                s = pre.tile([P, BC], f32, tag=f"s_{c}")
                nc.vector.tensor_mul(s[:], hb[:], sig[:])
                return (s, hb) if keep_hb else (s, None)

            def resmlp_T(wf_sb, b0s, b0u, w1_sb, b1s, b1u, w2_sb, b2u,
                         wo_sb, brow, flat_sb, tab_dram, pre, ppre, ptab):
                h0, s1, h1, s2, h2, s3, hb0 = [], [], [], [], [], [], []
                for c in (0, 1):
                    t_ = ppre.tile([P, BC], f32, tag=f"h0_{c}")
                    nc.tensor.matmul(t_[:], wf_sb[0:1, c * P:(c + 1) * P],
                                     ev_sb[:], start=True, stop=True)
                    h0.append(t_)
                for c in (0, 1):
                    s, hb = swishT(
                        f"a{c}", h0[c],
                        b0s[:, c:c + 1] if b0s is not None else None,
                        b0u[:, c:c + 1] if b0u is not None else None,
                        pre, keep_hb=True)
                    s1.append(s)
                    hb0.append(hb if hb is not None else h0[c])
                for m in (0, 1):
                    t_ = ppre.tile([P, BC], f32, tag=f"h1_{m}")
                    for k in (0, 1):
                        nc.tensor.matmul(t_[:], w1_sb[:, k, m, :], s1[k][:],
                                         start=(k == 0), stop=(k == 1))
                    h1.append(t_)
                for m in (0, 1):
                    s, _ = swishT(
                        f"b{m}", h1[m],
                        b1s[:, m:m + 1] if b1s is not None else None,
                        b1u[:, m:m + 1] if b1u is not None else None, pre)
                    s2.append(s)
                for m in (0, 1):
                    t_ = ppre.tile([P, BC], f32, tag=f"h2_{m}")
                    for k in (0, 1):
                        nc.tensor.matmul(t_[:], w2_sb[:, k, m, :], s2[k][:],
                                         start=(k == 0), stop=(k == 1))
                    h2.append(t_)
                for m in (0, 1):
                    rt = pre.tile([P, BC], f32, tag=f"r_{m}")
                    nc.vector.tensor_add(rt[:], hb0[m][:], h2[m][:])
                    if b2u is not None:
                        nc.vector.tensor_scalar_add(rt[:], rt[:],
                                                    b2u[:, m:m + 1])
                    sig = pre.tile([P, BC], f32, tag=f"sig3_{m}")
                    nc.scalar.activation(sig[:], rt[:], AF.Sigmoid, bias=0.0,
                                         scale=BETA)
                    s = pre.tile([P, BC], f32, tag=f"s3_{m}")
                    nc.vector.tensor_mul(s[:], rt[:], sig[:])
                    s3.append(s)
                tab = ptab.tile([BC, F], f32, tag="tab")
                for k in (0, 1):
                    nc.tensor.matmul(tab[:], s3[k][:], wo_sb[:, k, :],
                                     start=(k == 0),
                                     stop=(k == 1 and brow is None))
                if brow is not None:
                    nc.tensor.matmul(tab[:], ones1[:], brow[:], start=False,
                                     stop=True)
                tab_sb = pre.tile([BC, F], f32, tag="tab_sb")
                nc.vector.tensor_copy(tab_sb[:], tab[:])
                nc.sync.dma_start(out=tab_dram[:], in_=tab_sb[:])
                for q in range(2):
                    nc.sync.dma_start(
                        out=flat_sb[64 * q:64 * q + 1, :],
                        in_=tab_dram[32 * q:32 * (q + 1), :].rearrange(
                            "(a blk) f -> a (blk f)", a=1))

            with (
                tc.tile_pool(name="pre", bufs=2) as pre,
                tc.tile_pool(name="ppre", bufs=1, space="PSUM") as ppre,
                tc.tile_pool(name="ptab", bufs=1, space="PSUM") as ptab,
            ):
                resmlp_T(wkf_sb, bkfs_sb, bkfu_sb, kw1_sb, kb1s_sb,
                         kb1u_sb, kw2_sb, kb2u_sb, woqk_sb, bq_sb, kq_flat,
                         kqtab_d, pre, ppre, ptab)
                resmlp_T(wvf_sb, None, None, vw1_sb, None, None,
                         vw2_sb, None, wovv_sb, None, v_flat, vtab_d,
                         pre, ppre, ptab)

            with (
                tc.tile_pool(name="kqbc", bufs=3, space="PSUM") as kqp,
                tc.tile_pool(name="vbc", bufs=3, space="PSUM") as vbp,
                tc.tile_pool(name="pan", bufs=1, space="PSUM") as pan,
            ):
                # ---- pass 1: dot, exp, segment sums ----
                anorm = pan.tile([1, BC], f32)
                kq_tiles = {}
                for g in range(TU // G):
                    xg = xp.tile([P, G, F], f32, tag="xg")
                    nc.sync.dma_start(out=xg[:], in_=xv[:, g, :, :])
                    dotg = dp.tile([P, G], f32, tag="dotg")
                    for j in range(G):
                        t = g * G + j
                        b = t // KM
                        if t % KM == 0:
                            bt = kqp.tile([P, F], f32, tag="kqbc")
                            q, blk = b // 32, b % 32
                            nc.tensor.matmul(
                                bt[:], onesp[64 * q:64 * q + 1, :],
                                kq_flat[64 * q:64 * q + 1, blk * F:(blk + 1) * F],
                                start=True, stop=True)
                            kq_tiles[b] = bt
                        scr = scp.tile([P, F], f32, tag="scr")
                        nc.vector.scalar_tensor_tensor(
                            scr[:], xg[:, j, :], 1.0, kq_tiles[b][:],
                            ALU.mult, ALU.mult, accum_out=dotg[:, j:j + 1])
                    nc.scalar.activation(a_all[:, g * G:(g + 1) * G], dotg[:],
                                         AF.Exp, bias=0.0, scale=1.0 / SQRT_F)
                    # segment-sum every molecule completed by this group
                    for b in range((g * G) // KM, TU // KM):
                        last = b * KM + KM - 1
                        if not (g * G <= last < (g + 1) * G):
                            continue
                        apm = dp.tile([P, 1], f32, tag="apm")
                        s7 = scp.tile([P, KM], f32, tag="s7")
                        nc.vector.scalar_tensor_tensor(
                            s7[:], a_all[:, b * KM:(b + 1) * KM], 1.0,
                            pm_sb[:, b * KM:(b + 1) * KM],
                            ALU.mult, ALU.mult, accum_out=apm[:])
                        nc.tensor.matmul(anorm[0:1, b:b + 1], apm[:],
                                         onesp[:, 0:1], start=True, stop=True)

                # ---- interphase: r = 1/(anorm+eps), ahat = a*r[seg] ----
                an_eps = sg.tile([1, BC], f32)
                nc.vector.tensor_scalar_add(an_eps[:], anorm[:], EPS)
                rrec = sg.tile([1, BC], f32)
                nc.vector.reciprocal(rrec[:], an_eps[:])
                nc.gpsimd.partition_broadcast(r_bc[:], rrec[:])
                for b in range(BC):
                    nc.vector.tensor_scalar_mul(
                        ahat_all[:, b * KM:(b + 1) * KM],
                        a_all[:, b * KM:(b + 1) * KM], r_bc[:, b:b + 1])

                # ---- pass 2: out = ahat * v[seg] ----
                v_tiles = {}
                for g in range(TU // G):
                    og = op.tile([P, G, F], f32, tag="og")
                    for j in range(G):
                        t = g * G + j
                        b = t // KM
                        if t % KM == 0:
                            bt = vbp.tile([P, F], f32, tag="vbc")
                            q, blk = b // 32, b % 32
                            nc.tensor.matmul(
                                bt[:], onesp[64 * q:64 * q + 1, :],
                                v_flat[64 * q:64 * q + 1, blk * F:(blk + 1) * F],
                                start=True, stop=True)
                            v_tiles[b] = bt
                        nc.scalar.activation(og[:, j, :], v_tiles[b][:], AF.Copy,
                                             scale=ahat_all[:, t:t + 1])
                    nc.sync.dma_start(out=ov[:, g, :, :], in_=og[:])

    nc.compile()
    return nc


def _prep_host(x, E, batch_seg, Wq, Wkf, bkf, Wvf, kW1, kb1, kW2, kb2, kWo,
               kbo, vW1, vW2, vWo):
    f32 = np.float32
    bs = np.asarray(batch_seg).astype(np.int64)
    x = np.asarray(x, dtype=f32)
    N = x.shape[0]
    counts = np.bincount(bs, minlength=B)
    KM = int(max(1, -(-counts.max() // P)))
    TU = BC * KM

    mol_start = np.searchsorted(bs, np.arange(B), side="left")
    local_mol = bs % BC
    offs = np.arange(N) - mol_start[bs]
    t_idx = local_mol * KM + offs // P
    p_idx = offs % P
    dest_row = (t_idx // G) * (P * G) + p_idx * G + (t_idx % G)
    core_bounds = np.searchsorted(bs, np.arange(NCORES + 1) * BC, side="left")

    x_pads, pms, evs = [], [], []
    for c in range(NCORES):
        n0, n1 = core_bounds[c], core_bounds[c + 1]
        xp_ = np.zeros((TU * P, F), dtype=f32)
        xp_[dest_row[n0:n1]] = x[n0:n1]
        pm = np.zeros((P, TU), dtype=f32)
        pm[p_idx[n0:n1], t_idx[n0:n1]] = 1.0
        x_pads.append(xp_)
        pms.append(pm)
        evs.append(np.ascontiguousarray(
            np.asarray(E, dtype=f32)[c * BC:(c + 1) * BC].reshape(1, BC)))

    def pack_w(W):
        A = np.asarray(W, dtype=f32)
        return np.ascontiguousarray(A.reshape(2, P, 2, P).transpose(3, 2, 0, 1))

    def pack_b(v, scale):
        a = (np.asarray(v, dtype=f32) * f32(scale)).astype(f32)
        return np.ascontiguousarray(a.reshape(2, P).T)

    Wq_, kWo_, vWo_ = (np.asarray(a, dtype=f32) for a in (Wq, kWo, vWo))
    woq = (kWo_.T @ Wq_).astype(f32)
    wov = vWo_.T.astype(f32)
    weights = dict(
        wkf=np.ascontiguousarray(np.asarray(Wkf, dtype=f32).reshape(F)[None, :]),
        wvf=np.ascontiguousarray(np.asarray(Wvf, dtype=f32).reshape(F)[None, :]),
        kw1=pack_w(kW1), kw2=pack_w(kW2),
        vw1=pack_w(vW1), vw2=pack_w(vW2),
        woqk=np.ascontiguousarray(woq.reshape(2, P, F).transpose(1, 0, 2)),
        wovv=np.ascontiguousarray(wov.reshape(2, P, F).transpose(1, 0, 2)),
        bq=np.ascontiguousarray(
            (np.asarray(kbo, dtype=f32) @ Wq_).reshape(1, F)),
        bkfs=pack_b(bkf, BETA), bkfu=pack_b(bkf, 1.0),
        kb1s=pack_b(kb1, BETA), kb1u=pack_b(kb1, 1.0),
        kb2u=pack_b(kb2, 1.0),
    )
    return KM, x_pads, pms, evs, weights, dest_row, core_bounds


_CACHE = {}
LAST_RESULT = None


def kernel(x, E, num_batch, batch_seg, Wq, Wkf, bkf, Wvf, kW1, kb1, kW2, kb2,
           kWo, kbo, vW1, vW2, vWo, **_ignored):
    from concourse.bass_utils import run_bass_kernel_spmd

    KM, x_pads, pms, evs, weights, dest_row, core_bounds = _prep_host(
        x, E, batch_seg, Wq, Wkf, bkf, Wvf, kW1, kb1, kW2, kb2, kWo, kbo,
        vW1, vW2, vWo)

    if KM not in _CACHE:
        _CACHE[KM] = _build_program(KM)
    nc = _CACHE[KM]

    in_maps = [
        dict(weights, x=x_pads[c], pm=pms[c], ev=evs[c])
        for c in range(NCORES)
    ]
    res = run_bass_kernel_spmd(nc, in_maps, core_ids=list(range(NCORES)))
    global LAST_RESULT
    LAST_RESULT = res

    N = np.asarray(x).shape[0]
    out = np.empty((N, F), dtype=np.float32)
    for c in range(NCORES):
        n0, n1 = core_bounds[c], core_bounds[c + 1]
        out[n0:n1] = res.results[c]["out"][dest_row[n0:n1]]
    return out



# revision 2
# speedup vs baseline: 1.5566x; 1.5566x over previous
"""Trainium2 Bass kernel for NonlinearElectronicEmbedding (segment softmax).

Design ("T2", transposed / padding-free):
  - 512 molecules -> 64 consecutive molecules per core (8 cores). Atoms of
    a core's molecules form one contiguous run (batch_seg sorted); x is
    shipped TRANSPOSED (features on partitions, atoms on the free axis) in
    fp16, so there is no 128-atom padding at all.
  - Prelude computes the k/v tables from E via the ResidualMLPs in
    transposed layout (features on partitions), fusing Wq and kbo@Wq into
    the k-table:  dot(a) = x(a) . (k_mol @ Wq)[seg(a)].
  - Main loop over "supers" of 1024 atoms:
      dots  = kqT^T @ xT           (PE, all 64 molecules at once, fp16)
      e     = exp(dots/16)         (ACT, PSUM->SBUF fp16)
      S     = e * mask, partial = rowsum(S)   (DVE stt fused accum)
      anorm += partial; r = 1/(anorm+eps)     (tiny DVE)
      S[s-1] *= r  (per-partition scalar; every molecule of super s-1 is
                    closed by the end of super s since molecules < 1024)
      outT[s-1] = v16^T @ S[s-1]   (PE outer product, K=64)
      copy PSUM->SBUF fp16 (split ACT/DVE), DMA out.
  - mask is a host-built fp16 0/1 band matrix [64, NCpad] (bs sorted ->
    band). Garbage dot rows (wrong molecules) are zeroed by it; softmax
    shift invariance makes the seg_max pass unnecessary (args bounded).
  - Host does only layout work: transpose+fp16 cast in, transpose+fp32
    cast out.
HBM traffic/core ~ 26+6+26 MB (x + mask + out, fp16) -> memory roofline.
"""

import numpy as np

F = 256
B = 512
NCORES = 8
BC = B // NCORES  # molecules per core
P = 128
SUP = 1024        # atoms per super-group (2 PSUM banks of dots)
HB = SUP // 2     # 512, one PSUM bank
BETA = 1.702
EPS = 1e-8
INV_SQRT_F = 1.0 / 16.0


def _build_program(nsup):
    import concourse.bacc as bacc
    import concourse.mybir as mybir
    import concourse.tile as tile

    dt = mybir.dt
    f32 = dt.float32
    f16 = dt.float16
    AF = mybir.ActivationFunctionType
    ALU = mybir.AluOpType

    NCpad = nsup * SUP

    nc = bacc.Bacc(trn_type="TRN2")

    x_h = nc.dram_tensor("x", [2 * P, NCpad], f16, kind="ExternalInput")
    mk_h = nc.dram_tensor("mk", [BC, NCpad], f16, kind="ExternalInput")
    ev_h = nc.dram_tensor("ev", [1, BC], f32, kind="ExternalInput")
    wkf_h = nc.dram_tensor("wkf", [1, F], f32, kind="ExternalInput")
    wvf_h = nc.dram_tensor("wvf", [1, F], f32, kind="ExternalInput")
    kw1_h = nc.dram_tensor("kw1", [P, 2, 2, P], f32, kind="ExternalInput")
    kw2_h = nc.dram_tensor("kw2", [P, 2, 2, P], f32, kind="ExternalInput")
    vw1_h = nc.dram_tensor("vw1", [P, 2, 2, P], f32, kind="ExternalInput")
    vw2_h = nc.dram_tensor("vw2", [P, 2, 2, P], f32, kind="ExternalInput")
    woqk_h = nc.dram_tensor("woqk", [P, 2, 2, P], f32, kind="ExternalInput")
    wovv_h = nc.dram_tensor("wovv", [P, 2, 2, P], f32, kind="ExternalInput")
    bq_h = nc.dram_tensor("bq", [1, F], f32, kind="ExternalInput")
    # biases: [P, 2] chunked; *_s pre-multiplied by BETA, *_u raw
    bkfs_h = nc.dram_tensor("bkfs", [P, 2], f32, kind="ExternalInput")
    bkfu_h = nc.dram_tensor("bkfu", [P, 2], f32, kind="ExternalInput")
    kb1s_h = nc.dram_tensor("kb1s", [P, 2], f32, kind="ExternalInput")
    kb1u_h = nc.dram_tensor("kb1u", [P, 2], f32, kind="ExternalInput")
    kb2u_h = nc.dram_tensor("kb2u", [P, 2], f32, kind="ExternalInput")
    out_h = nc.dram_tensor("out", [2 * P, NCpad], f16, kind="ExternalOutput")

    # DRAM views with feature-chunk as a middle axis so one DMA moves both
    # 128-row chunks into/out of a [128, 2, SUP] SBUF tile
    xv = x_h[:].rearrange("(c p) j -> p c j", c=2)
    ov = out_h[:].rearrange("(c p) j -> p c j", c=2)

    with tile.TileContext(nc) as tc:
        with (
            tc.tile_pool(name="singles", bufs=1) as sg,
            tc.tile_pool(name="xpool", bufs=3) as xp,
            tc.tile_pool(name="mpool", bufs=3) as mp,
            tc.tile_pool(name="epool", bufs=2) as ep,
            tc.tile_pool(name="spool", bufs=3) as sp_,
            tc.tile_pool(name="opool", bufs=3) as op,
            tc.tile_pool(name="rpool", bufs=3) as rp,
        ):
            def load(name, h, shape):
                t_ = sg.tile(shape, f32, tag=name, name=name)
                nc.sync.dma_start(out=t_[:], in_=h[:])
                return t_

            ev_sb = load("ev", ev_h, [1, BC])
            wkf_sb = load("wkf", wkf_h, [1, F])
            wvf_sb = load("wvf", wvf_h, [1, F])
            kw1_sb = load("kw1", kw1_h, [P, 2, 2, P])
            kw2_sb = load("kw2", kw2_h, [P, 2, 2, P])
            vw1_sb = load("vw1", vw1_h, [P, 2, 2, P])
            vw2_sb = load("vw2", vw2_h, [P, 2, 2, P])
            woqk_sb = load("woqk", woqk_h, [P, 2, 2, P])
            wovv_sb = load("wovv", wovv_h, [P, 2, 2, P])
            bq_sb = load("bq", bq_h, [1, F])
            bkfs_sb = load("bkfs", bkfs_h, [P, 2])
            bkfu_sb = load("bkfu", bkfu_h, [P, 2])
            kb1s_sb = load("kb1s", kb1s_h, [P, 2])
            kb1u_sb = load("kb1u", kb1u_h, [P, 2])
            kb2u_sb = load("kb2u", kb2u_h, [P, 2])

            ones1 = sg.tile([1, BC], f32)
            nc.vector.memset(ones1[:], 1.0)

            kqT16 = sg.tile([P, 2, BC], f16)   # kqT16[f', c, b]
            v16 = sg.tile([BC, 2, P], f16)     # v16[b, c, f']
            anorm_run = sg.tile([BC, 1], f32)
            nc.vector.memset(anorm_run[:], 0.0)

            # ---- prelude: ResidualMLP in transposed layout ----
            # swish(y) = y * sigmoid(BETA*y); h_psum holds y - b.
            def swishT(c, h_psum, bs_ap, bu_ap, pre, keep_hb=False):
                sig = pre.tile([P, BC], f32, tag=f"sig_{c}", name="sig")
                nc.scalar.activation(sig[:], h_psum[:], AF.Sigmoid,
                                     bias=bs_ap if bs_ap is not None else 0.0,
                                     scale=BETA)
                if bu_ap is not None:
                    hb = pre.tile([P, BC], f32, tag=f"hb_{c}", name="hb")
                    nc.vector.tensor_scalar_add(hb[:], h_psum[:], bu_ap)
                elif keep_hb:
                    hb = pre.tile([P, BC], f32, tag=f"hb_{c}", name="hb")
                    nc.vector.tensor_copy(hb[:], h_psum[:])
                else:
                    hb = h_psum
                s = pre.tile([P, BC], f32, tag=f"s_{c}", name="s")
                nc.vector.tensor_mul(s[:], hb[:], sig[:])
                return (s, hb) if keep_hb else (s, None)

            def resmlp_T(wf_sb, b0s, b0u, w1_sb, b1s, b1u, w2_sb, b2u,
                         pre, ppre, branch):
                h0, s1, h1, s2, h2, s3, hb0 = [], [], [], [], [], [], []
                for c in (0, 1):
                    t_ = ppre.tile([P, BC], f32, tag=f"h0_{c}", name="h0")
                    nc.tensor.matmul(t_[:], wf_sb[0:1, c * P:(c + 1) * P],
                                     ev_sb[:], start=True, stop=True)
                    h0.append(t_)
                for c in (0, 1):
                    s, hb = swishT(
                        f"a{c}", h0[c],
                        b0s[:, c:c + 1] if b0s is not None else None,
                        b0u[:, c:c + 1] if b0u is not None else None,
                        pre, keep_hb=True)
                    s1.append(s)
                    hb0.append(hb if hb is not None else h0[c])
                for m in (0, 1):
                    t_ = ppre.tile([P, BC], f32, tag=f"h1_{m}", name="h1")
                    for k in (0, 1):
                        nc.tensor.matmul(t_[:], w1_sb[:, k, m, :], s1[k][:],
                                         start=(k == 0), stop=(k == 1))
                    h1.append(t_)
                for m in (0, 1):
                    s, _ = swishT(
                        f"b{m}", h1[m],
                        b1s[:, m:m + 1] if b1s is not None else None,
                        b1u[:, m:m + 1] if b1u is not None else None, pre)
                    s2.append(s)
                for m in (0, 1):
                    t_ = ppre.tile([P, BC], f32, tag=f"h2_{m}", name="h2")
                    for k in (0, 1):
                        nc.tensor.matmul(t_[:], w2_sb[:, k, m, :], s2[k][:],
                                         start=(k == 0), stop=(k == 1))
                    h2.append(t_)
                for m in (0, 1):
                    rt = pre.tile([P, BC], f32, tag=f"r_{m}_{branch}", name="rt")
                    nc.vector.tensor_add(rt[:], hb0[m][:], h2[m][:])
                    if b2u is not None:
                        nc.vector.tensor_scalar_add(rt[:], rt[:],
                                                    b2u[:, m:m + 1])
                    sig = pre.tile([P, BC], f32, tag=f"sig3_{m}", name="sig3")
                    nc.scalar.activation(sig[:], rt[:], AF.Sigmoid, bias=0.0,
                                         scale=BETA)
                    s = pre.tile([P, BC], f32, tag=f"s3_{m}_{branch}", name="s3")
                    nc.vector.tensor_mul(s[:], rt[:], sig[:])
                    s3.append(s)
                return s3

            with (
                tc.tile_pool(name="pre", bufs=2) as pre,
                tc.tile_pool(name="ppre", bufs=1, space="PSUM") as ppre,
                tc.tile_pool(name="ptab", bufs=1, space="PSUM") as ptab,
            ):
                s3k = resmlp_T(wkf_sb, bkfs_sb, bkfu_sb, kw1_sb, kb1s_sb,
                               kb1u_sb, kw2_sb, kb2u_sb, pre, ppre, "k")
                s3v = resmlp_T(wvf_sb, None, None, vw1_sb, None, None,
                               vw2_sb, None, pre, ppre, "v")
                # kqT[g, b] = sum_h s3k[h, b] * woq[h, g] + bq[g]
                pkq = ptab.tile([P, 2, BC], f32, tag="pkq")
                for c in (0, 1):
                    nc.tensor.matmul(pkq[:, c, :], woqk_sb[:, 0, c, :],
                                     s3k[0][:], start=True, stop=False)
                    nc.tensor.matmul(pkq[:, c, :], woqk_sb[:, 1, c, :],
                                     s3k[1][:], start=False, stop=False)
                    nc.tensor.matmul(pkq[:, c, :],
                                     bq_sb[0:1, c * P:(c + 1) * P],
                                     ones1[:], start=False, stop=True)
                nc.vector.tensor_copy(kqT16[:], pkq[:])
                # v16[b, f'] (chunked) = sum_h s3v[h, b] * wov[h, f']
                pv = ptab.tile([BC, 2, P], f32, tag="pv")
                for c in (0, 1):
                    for k in (0, 1):
                        nc.tensor.matmul(pv[:, c, :], s3v[k][:],
                                         wovv_sb[:, k, c, :],
                                         start=(k == 0), stop=(k == 1))
                nc.vector.tensor_copy(v16[:], pv[:])

            with (
                tc.tile_pool(name="pdot", bufs=2, space="PSUM") as pd_pool,
                tc.tile_pool(name="pout", bufs=4, space="PSUM") as po_pool,
            ):
                S_tiles = [None] * nsup
                r_tiles = [None] * nsup

                def pass2(s):
                    # S[s] *= r (all molecules of super s closed by now)
                    St = S_tiles[s]
                    rt = r_tiles[min(s + 1, nsup - 1)]
                    nc.vector.tensor_scalar_mul(St[:], St[:], rt[:])
                    out16 = op.tile([P, 2, SUP], f16, tag="out16")
                    for c in (0, 1):
                        for b in (0, 1):
                            po = po_pool.tile([P, HB], f32, tag="po")
                            nc.tensor.matmul(
                                po[:], v16[:, c, :],
                                St[:, b * HB:(b + 1) * HB],
                                start=True, stop=True)
                            dst = out16[:, c, b * HB:(b + 1) * HB]
                            if c == 0:
                                nc.scalar.activation(dst, po[:], AF.Copy)
                            else:
                                nc.vector.tensor_copy(dst, po[:])
                    nc.sync.dma_start(
                        out=ov[:, :, s * SUP:(s + 1) * SUP], in_=out16[:])

                for s in range(nsup):
                    cols = slice(s * SUP, (s + 1) * SUP)
                    x16 = xp.tile([P, 2, SUP], f16, tag="x16")
                    nc.sync.dma_start(out=x16[:], in_=xv[:, :, cols])
                    mk = mp.tile([BC, SUP], f16, tag="mk")
                    nc.sync.dma_start(out=mk[:], in_=mk_h[:, cols])

                    pd = pd_pool.tile([BC, 2, HB], f32, tag="pd")
                    for c in (0, 1):
                        for b in (0, 1):
                            nc.tensor.matmul(
                                pd[:, b, :], kqT16[:, c, :],
                                x16[:, c, b * HB:(b + 1) * HB],
                                start=(c == 0), stop=(c == 1),
                                skip_group_check=True)
                    e16 = ep.tile([BC, SUP], f16, tag="e16")
                    for b in (0, 1):
                        nc.scalar.activation(
                            e16[:, b * HB:(b + 1) * HB], pd[:, b, :],
                            AF.Exp, bias=0.0, scale=INV_SQRT_F)
                    St = sp_.tile([BC, SUP], f16, tag="St")
                    part = rp.tile([BC, 1], f32, tag="part")
                    nc.vector.scalar_tensor_tensor(
                        St[:], e16[:], 1.0, mk[:], ALU.mult, ALU.mult,
                        accum_out=part[:])
                    S_tiles[s] = St
                    nc.vector.tensor_add(anorm_run[:], anorm_run[:], part[:])
                    rt = rp.tile([BC, 1], f32, tag="rt")
                    nc.vector.tensor_scalar_add(rt[:], anorm_run[:], EPS)
                    nc.vector.reciprocal(rt[:], rt[:])
                    r_tiles[s] = rt
                    if s >= 1:
                        pass2(s - 1)
                pass2(nsup - 1)

    nc.compile()
    return nc


def _prep_host(x, E, batch_seg, Wq, Wkf, bkf, Wvf, kW1, kb1, kW2, kb2, kWo,
               kbo, vW1, vW2, vWo):
    f32 = np.float32
    f16 = np.float16
    bs = np.asarray(batch_seg).astype(np.int64)
    x = np.asarray(x, dtype=f32)
    N = x.shape[0]
    core_bounds = np.searchsorted(bs, np.arange(NCORES + 1) * BC, side="left")
    NCmax = int(np.max(np.diff(core_bounds)))
    nsup = max(1, -(-NCmax // SUP))
    NCpad = nsup * SUP

    xts, mks, evs = [], [], []
    E32 = np.asarray(E, dtype=f32)
    for c in range(NCORES):
        n0, n1 = core_bounds[c], core_bounds[c + 1]
        nc_ = n1 - n0
        xt = np.zeros((2 * P, NCpad), dtype=f16)
        xt[:, :nc_] = x[n0:n1].T.astype(f16)
        mk = np.zeros((BC, NCpad), dtype=f16)
        mk[:, :nc_] = (bs[n0:n1][None, :]
                       == (np.arange(BC) + c * BC)[:, None]).astype(f16)
        xts.append(xt)
        mks.append(mk)
        evs.append(np.ascontiguousarray(E32[c * BC:(c + 1) * BC].reshape(1, BC)))

    def pack_w(W):
        A = np.asarray(W, dtype=f32)
        return np.ascontiguousarray(A.reshape(2, P, 2, P).transpose(3, 2, 0, 1))

    def pack_hw(M):
        # M [F(h), F(g)] -> [P(h'), k(h-half), c(g-half), P(g')]
        return np.ascontiguousarray(
            M.reshape(2, P, 2, P).transpose(1, 0, 2, 3))

    def pack_b(v, scale):
        a = (np.asarray(v, dtype=f32) * f32(scale)).astype(f32)
        return np.ascontiguousarray(a.reshape(2, P).T)

    Wq_, kWo_, vWo_ = (np.asarray(a, dtype=f32) for a in (Wq, kWo, vWo))
    woq = (kWo_.T @ Wq_).astype(f32)   # [h, g]
    wov = vWo_.T.astype(f32)           # [h, f]
    weights = dict(
        wkf=np.ascontiguousarray(np.asarray(Wkf, dtype=f32).reshape(F)[None, :]),
        wvf=np.ascontiguousarray(np.asarray(Wvf, dtype=f32).reshape(F)[None, :]),
        kw1=pack_w(kW1), kw2=pack_w(kW2),
        vw1=pack_w(vW1), vw2=pack_w(vW2),
        woqk=pack_hw(woq), wovv=pack_hw(wov),
        bq=np.ascontiguousarray(
            (np.asarray(kbo, dtype=f32) @ Wq_).reshape(1, F)),
        bkfs=pack_b(bkf, BETA), bkfu=pack_b(bkf, 1.0),
        kb1s=pack_b(kb1, BETA), kb1u=pack_b(kb1, 1.0),
        kb2u=pack_b(kb2, 1.0),
    )
    return nsup, xts, mks, evs, weights, core_bounds


_CACHE = {}
LAST_RESULT = None


def kernel(x, E, num_batch, batch_seg, Wq, Wkf, bkf, Wvf, kW1, kb1, kW2, kb2,
           kWo, kbo, vW1, vW2, vWo, **_ignored):
    from concourse.bass_utils import run_bass_kernel_spmd

    nsup, xts, mks, evs, weights, core_bounds = _prep_host(
        x, E, batch_seg, Wq, Wkf, bkf, Wvf, kW1, kb1, kW2, kb2, kWo, kbo,
        vW1, vW2, vWo)

    if nsup not in _CACHE:
        _CACHE[nsup] = _build_program(nsup)
    nc = _CACHE[nsup]

    in_maps = [
        dict(weights, x=xts[c], mk=mks[c], ev=evs[c])
        for c in range(NCORES)
    ]
    res = run_bass_kernel_spmd(nc, in_maps, core_ids=list(range(NCORES)))
    global LAST_RESULT
    LAST_RESULT = res

    N = np.asarray(x).shape[0]
    out = np.empty((N, F), dtype=np.float32)
    for c in range(NCORES):
        n0, n1 = core_bounds[c], core_bounds[c + 1]
        out[n0:n1] = res.results[c]["out"][:, :n1 - n0].T.astype(np.float32)
    return out


# revision 4
# speedup vs baseline: 1.6360x; 1.0511x over previous
"""Trainium2 Bass kernel for NonlinearElectronicEmbedding (segment softmax).

Design ("T2", transposed / padding-free):
  - 512 molecules -> 64 consecutive molecules per core (8 cores). Atoms of
    a core's molecules form one contiguous run (batch_seg sorted); x is
    shipped TRANSPOSED (features on partitions, atoms on the free axis) in
    fp16, so there is no 128-atom padding at all.
  - Prelude computes the k/v tables from E via the ResidualMLPs in
    transposed layout (features on partitions), fusing Wq and kbo@Wq into
    the k-table:  dot(a) = x(a) . (k_mol @ Wq)[seg(a)].
  - Main loop over "supers" of 1024 atoms:
      dots  = kqT^T @ xT           (PE, all 64 molecules at once, fp16)
      e     = exp(dots/16)         (ACT, PSUM->SBUF fp16)
      S     = e * mask, partial = rowsum(S)   (DVE stt fused accum)
      anorm += partial; r = 1/(anorm+eps)     (tiny DVE)
      S[s-1] *= r  (per-partition scalar; every molecule of super s-1 is
                    closed by the end of super s since molecules < 1024)
      outT[s-1] = v16^T @ S[s-1]   (PE outer product, K=64)
      copy PSUM->SBUF fp16 (split ACT/DVE), DMA out.
  - mask is a host-built fp16 0/1 band matrix [64, NCpad] (bs sorted ->
    band). Garbage dot rows (wrong molecules) are zeroed by it; softmax
    shift invariance makes the seg_max pass unnecessary (args bounded).
  - Host does only layout work: transpose+fp16 cast in, transpose+fp32
    cast out.
HBM traffic/core ~ 26+6+26 MB (x + mask + out, fp16) -> memory roofline.
"""

import numpy as np

F = 256
B = 512
NCORES = 8
BC = B // NCORES  # molecules per core
P = 128
SUP = 1024        # atoms per super-group (2 PSUM banks of dots)
HB = SUP // 2     # 512, one PSUM bank
BETA = 1.702
EPS = 1e-8
INV_SQRT_F = 1.0 / 16.0


def _build_program(nsup):
    import concourse.bacc as bacc
    import concourse.mybir as mybir
    import concourse.tile as tile

    dt = mybir.dt
    f32 = dt.float32
    f16 = dt.float16
    AF = mybir.ActivationFunctionType
    ALU = mybir.AluOpType

    NCpad = nsup * SUP

    nc = bacc.Bacc(trn_type="TRN2")

    x_h = nc.dram_tensor("x", [2 * P, NCpad], f16, kind="ExternalInput")
    mk_h = nc.dram_tensor("mk", [BC, NCpad], f16, kind="ExternalInput")
    ev_h = nc.dram_tensor("ev", [1, BC], f32, kind="ExternalInput")
    wkf_h = nc.dram_tensor("wkf", [1, F], f32, kind="ExternalInput")
    wvf_h = nc.dram_tensor("wvf", [1, F], f32, kind="ExternalInput")
    kw1_h = nc.dram_tensor("kw1", [P, 2, 2, P], f32, kind="ExternalInput")
    kw2_h = nc.dram_tensor("kw2", [P, 2, 2, P], f32, kind="ExternalInput")
    vw1_h = nc.dram_tensor("vw1", [P, 2, 2, P], f32, kind="ExternalInput")
    vw2_h = nc.dram_tensor("vw2", [P, 2, 2, P], f32, kind="ExternalInput")
    woqk_h = nc.dram_tensor("woqk", [P, 2, 2, P], f32, kind="ExternalInput")
    wovv_h = nc.dram_tensor("wovv", [P, 2, 2, P], f32, kind="ExternalInput")
    bq_h = nc.dram_tensor("bq", [1, F], f32, kind="ExternalInput")
    # biases: [P, 2] chunked; *_s pre-multiplied by BETA, *_u raw
    bkfs_h = nc.dram_tensor("bkfs", [P, 2], f32, kind="ExternalInput")
    bkfu_h = nc.dram_tensor("bkfu", [P, 2], f32, kind="ExternalInput")
    kb1s_h = nc.dram_tensor("kb1s", [P, 2], f32, kind="ExternalInput")
    kb1u_h = nc.dram_tensor("kb1u", [P, 2], f32, kind="ExternalInput")
    kb2u_h = nc.dram_tensor("kb2u", [P, 2], f32, kind="ExternalInput")
    out_h = nc.dram_tensor("out", [2 * P, NCpad], f16, kind="ExternalOutput")

    # DRAM views with feature-chunk as a middle axis so one DMA moves both
    # 128-row chunks into/out of a [128, 2, SUP] SBUF tile
    xv = x_h[:].rearrange("(c p) j -> p c j", c=2)
    ov = out_h[:].rearrange("(c p) j -> p c j", c=2)

    with tile.TileContext(nc) as tc:
        with (
            tc.tile_pool(name="singles", bufs=1) as sg,
            tc.tile_pool(name="xpool", bufs=4) as xp,
            tc.tile_pool(name="mpool", bufs=4) as mp,
            tc.tile_pool(name="epool", bufs=2) as ep,
            tc.tile_pool(name="spool", bufs=4) as sp_,
            tc.tile_pool(name="opool", bufs=3) as op,
            tc.tile_pool(name="rpool", bufs=4) as rp,
        ):
            def load(name, h, shape):
                t_ = sg.tile(shape, f32, tag=name, name=name)
                nc.sync.dma_start(out=t_[:], in_=h[:])
                return t_

            ev_sb = load("ev", ev_h, [1, BC])
            wkf_sb = load("wkf", wkf_h, [1, F])
            wvf_sb = load("wvf", wvf_h, [1, F])
            kw1_sb = load("kw1", kw1_h, [P, 2, 2, P])
            kw2_sb = load("kw2", kw2_h, [P, 2, 2, P])
            vw1_sb = load("vw1", vw1_h, [P, 2, 2, P])
            vw2_sb = load("vw2", vw2_h, [P, 2, 2, P])
            woqk_sb = load("woqk", woqk_h, [P, 2, 2, P])
            wovv_sb = load("wovv", wovv_h, [P, 2, 2, P])
            bq_sb = load("bq", bq_h, [1, F])
            bkfs_sb = load("bkfs", bkfs_h, [P, 2])
            bkfu_sb = load("bkfu", bkfu_h, [P, 2])
            kb1s_sb = load("kb1s", kb1s_h, [P, 2])
            kb1u_sb = load("kb1u", kb1u_h, [P, 2])
            kb2u_sb = load("kb2u", kb2u_h, [P, 2])

            ones1 = sg.tile([1, BC], f32)
            nc.vector.memset(ones1[:], 1.0)

            kqT16 = sg.tile([P, 2, BC], f16)   # kqT16[f', c, b]
            v16 = sg.tile([BC, 2, P], f16)     # v16[b, c, f']
            anorm_run = sg.tile([BC, 1], f32)
            nc.vector.memset(anorm_run[:], 0.0)

            # ---- prelude: ResidualMLP in transposed layout ----
            # swish(y) = y * sigmoid(BETA*y); h_psum holds y - b.
            def swishT(c, h_psum, bs_ap, bu_ap, pre, keep_hb=False):
                sig = pre.tile([P, BC], f32, tag=f"sig_{c}", name="sig")
                nc.scalar.activation(sig[:], h_psum[:], AF.Sigmoid,
                                     bias=bs_ap if bs_ap is not None else 0.0,
                                     scale=BETA)
                if bu_ap is not None:
                    hb = pre.tile([P, BC], f32, tag=f"hb_{c}", name="hb")
                    nc.vector.tensor_scalar_add(hb[:], h_psum[:], bu_ap)
                elif keep_hb:
                    hb = pre.tile([P, BC], f32, tag=f"hb_{c}", name="hb")
                    nc.vector.tensor_copy(hb[:], h_psum[:])
                else:
                    hb = h_psum
                s = pre.tile([P, BC], f32, tag=f"s_{c}", name="s")
                nc.vector.tensor_mul(s[:], hb[:], sig[:])
                return (s, hb) if keep_hb else (s, None)

            def resmlp_T(wf_sb, b0s, b0u, w1_sb, b1s, b1u, w2_sb, b2u,
                         pre, ppre, branch):
                h0, s1, h1, s2, h2, s3, hb0 = [], [], [], [], [], [], []
                for c in (0, 1):
                    t_ = ppre.tile([P, BC], f32, tag=f"h0_{c}", name="h0")
                    nc.tensor.matmul(t_[:], wf_sb[0:1, c * P:(c + 1) * P],
                                     ev_sb[:], start=True, stop=True)
                    h0.append(t_)
                for c in (0, 1):
                    s, hb = swishT(
                        f"a{c}", h0[c],
                        b0s[:, c:c + 1] if b0s is not None else None,
                        b0u[:, c:c + 1] if b0u is not None else None,
                        pre, keep_hb=True)
                    s1.append(s)
                    hb0.append(hb if hb is not None else h0[c])
                for m in (0, 1):
                    t_ = ppre.tile([P, BC], f32, tag=f"h1_{m}", name="h1")
                    for k in (0, 1):
                        nc.tensor.matmul(t_[:], w1_sb[:, k, m, :], s1[k][:],
                                         start=(k == 0), stop=(k == 1))
                    h1.append(t_)
                for m in (0, 1):
                    s, _ = swishT(
                        f"b{m}", h1[m],
                        b1s[:, m:m + 1] if b1s is not None else None,
                        b1u[:, m:m + 1] if b1u is not None else None, pre)
                    s2.append(s)
                for m in (0, 1):
                    t_ = ppre.tile([P, BC], f32, tag=f"h2_{m}", name="h2")
                    for k in (0, 1):
                        nc.tensor.matmul(t_[:], w2_sb[:, k, m, :], s2[k][:],
                                         start=(k == 0), stop=(k == 1))
                    h2.append(t_)
                for m in (0, 1):
                    rt = pre.tile([P, BC], f32, tag=f"r_{m}_{branch}", name="rt")
                    nc.vector.tensor_add(rt[:], hb0[m][:], h2[m][:])
                    if b2u is not None:
                        nc.vector.tensor_scalar_add(rt[:], rt[:],
                                                    b2u[:, m:m + 1])
                    sig = pre.tile([P, BC], f32, tag=f"sig3_{m}", name="sig3")
                    nc.scalar.activation(sig[:], rt[:], AF.Sigmoid, bias=0.0,
                                         scale=BETA)
                    s = pre.tile([P, BC], f32, tag=f"s3_{m}_{branch}", name="s3")
                    nc.vector.tensor_mul(s[:], rt[:], sig[:])
                    s3.append(s)
                return s3

            with (
                tc.tile_pool(name="pre", bufs=2) as pre,
                tc.tile_pool(name="ppre", bufs=1, space="PSUM") as ppre,
                tc.tile_pool(name="ptab", bufs=1, space="PSUM") as ptab,
            ):
                s3k = resmlp_T(wkf_sb, bkfs_sb, bkfu_sb, kw1_sb, kb1s_sb,
                               kb1u_sb, kw2_sb, kb2u_sb, pre, ppre, "k")
                s3v = resmlp_T(wvf_sb, None, None, vw1_sb, None, None,
                               vw2_sb, None, pre, ppre, "v")
                # kqT[g, b] = sum_h s3k[h, b] * woq[h, g] + bq[g]
                pkq = ptab.tile([P, 2, BC], f32, tag="pkq")
                for c in (0, 1):
                    nc.tensor.matmul(pkq[:, c, :], woqk_sb[:, 0, c, :],
                                     s3k[0][:], start=True, stop=False)
                    nc.tensor.matmul(pkq[:, c, :], woqk_sb[:, 1, c, :],
                                     s3k[1][:], start=False, stop=False)
                    nc.tensor.matmul(pkq[:, c, :],
                                     bq_sb[0:1, c * P:(c + 1) * P],
                                     ones1[:], start=False, stop=True)
                nc.vector.tensor_copy(kqT16[:], pkq[:])
                # v16[b, f'] (chunked) = sum_h s3v[h, b] * wov[h, f']
                pv = ptab.tile([BC, 2, P], f32, tag="pv")
                for c in (0, 1):
                    for k in (0, 1):
                        nc.tensor.matmul(pv[:, c, :], s3v[k][:],
                                         wovv_sb[:, k, c, :],
                                         start=(k == 0), stop=(k == 1))
                nc.vector.tensor_copy(v16[:], pv[:])

            with (
                tc.tile_pool(name="pdot", bufs=2, space="PSUM") as pd_pool,
                tc.tile_pool(name="pout", bufs=2, space="PSUM") as po_pool,
            ):
                S_tiles = [None] * nsup
                r_tiles = [None] * nsup

                def pass2(s):
                    # S[s] *= r (all molecules of super s closed by now)
                    St = S_tiles[s]
                    rt = r_tiles[min(s + 1, nsup - 1)]
                    nc.vector.tensor_scalar_mul(St[:], St[:], rt[:])
                    out16 = op.tile([P, 2, SUP], f16, tag="out16")
                    for c in (0, 1):
                        po = po_pool.tile([P, 2, HB], f32, tag="po")
                        for b in (0, 1):
                            nc.tensor.matmul(
                                po[:, b, :], v16[:, c, :],
                                St[:, b * HB:(b + 1) * HB],
                                start=True, stop=True)
                        dst = out16[:, c, :].rearrange("p (b j) -> p b j", b=2)
                        # alternate copy engine to balance ACT/DVE load
                        if (s + c) % 2 == 0:
                            nc.scalar.activation(dst, po[:], AF.Copy)
                        else:
                            nc.vector.tensor_copy(dst, po[:])
                    nc.sync.dma_start(
                        out=ov[:, :, s * SUP:(s + 1) * SUP], in_=out16[:])

                for s in range(nsup):
                    cols = slice(s * SUP, (s + 1) * SUP)
                    x16 = xp.tile([P, 2, SUP], f16, tag="x16")
                    nc.sync.dma_start(out=x16[:], in_=xv[:, :, cols])
                    mk = mp.tile([BC, SUP], f16, tag="mk")
                    nc.sync.dma_start(out=mk[:], in_=mk_h[:, cols])

                    pd = pd_pool.tile([BC, 2, HB], f32, tag="pd")
                    for c in (0, 1):
                        for b in (0, 1):
                            nc.tensor.matmul(
                                pd[:, b, :], kqT16[:, c, :],
                                x16[:, c, b * HB:(b + 1) * HB],
                                start=(c == 0), stop=(c == 1),
                                skip_group_check=True)
                    e16 = ep.tile([BC, SUP], f16, tag="e16")
                    nc.scalar.activation(
                        e16[:].rearrange("p (b j) -> p b j", b=2), pd[:],
                        AF.Exp, bias=0.0, scale=INV_SQRT_F)
                    St = sp_.tile([BC, SUP], f16, tag="St")
                    part = rp.tile([BC, 1], f32, tag="part")
                    nc.vector.scalar_tensor_tensor(
                        St[:], e16[:], 1.0, mk[:], ALU.mult, ALU.mult,
                        accum_out=part[:])
                    S_tiles[s] = St
                    nc.vector.tensor_add(anorm_run[:], anorm_run[:], part[:])
                    rt = rp.tile([BC, 1], f32, tag="rt")
                    nc.vector.tensor_scalar_add(rt[:], anorm_run[:], EPS)
                    nc.vector.reciprocal(rt[:], rt[:])
                    r_tiles[s] = rt
                    if s >= 2:
                        pass2(s - 2)
                pass2(nsup - 2)
                pass2(nsup - 1)

    nc.compile()
    return nc


def _prep_host(x, E, batch_seg, Wq, Wkf, bkf, Wvf, kW1, kb1, kW2, kb2, kWo,
               kbo, vW1, vW2, vWo):
    f32 = np.float32
    f16 = np.float16
    bs = np.asarray(batch_seg).astype(np.int64)
    x = np.asarray(x, dtype=f32)
    N = x.shape[0]
    core_bounds = np.searchsorted(bs, np.arange(NCORES + 1) * BC, side="left")
    NCmax = int(np.max(np.diff(core_bounds)))
    nsup = max(1, -(-NCmax // SUP))
    NCpad = nsup * SUP

    xts, mks, evs = [], [], []
    E32 = np.asarray(E, dtype=f32)
    for c in range(NCORES):
        n0, n1 = core_bounds[c], core_bounds[c + 1]
        nc_ = n1 - n0
        xt = np.zeros((2 * P, NCpad), dtype=f16)
        xt[:, :nc_] = x[n0:n1].T.astype(f16)
        mk = np.zeros((BC, NCpad), dtype=f16)
        mk[:, :nc_] = (bs[n0:n1][None, :]
                       == (np.arange(BC) + c * BC)[:, None]).astype(f16)
        xts.append(xt)
        mks.append(mk)
        evs.append(np.ascontiguousarray(E32[c * BC:(c + 1) * BC].reshape(1, BC)))

    def pack_w(W):
        A = np.asarray(W, dtype=f32)
        return np.ascontiguousarray(A.reshape(2, P, 2, P).transpose(3, 2, 0, 1))

    def pack_hw(M):
        # M [F(h), F(g)] -> [P(h'), k(h-half), c(g-half), P(g')]
        return np.ascontiguousarray(
            M.reshape(2, P, 2, P).transpose(1, 0, 2, 3))

    def pack_b(v, scale):
        a = (np.asarray(v, dtype=f32) * f32(scale)).astype(f32)
        return np.ascontiguousarray(a.reshape(2, P).T)

    Wq_, kWo_, vWo_ = (np.asarray(a, dtype=f32) for a in (Wq, kWo, vWo))
    woq = (kWo_.T @ Wq_).astype(f32)   # [h, g]
    wov = vWo_.T.astype(f32)           # [h, f]
    weights = dict(
        wkf=np.ascontiguousarray(np.asarray(Wkf, dtype=f32).reshape(F)[None, :]),
        wvf=np.ascontiguousarray(np.asarray(Wvf, dtype=f32).reshape(F)[None, :]),
        kw1=pack_w(kW1), kw2=pack_w(kW2),
        vw1=pack_w(vW1), vw2=pack_w(vW2),
        woqk=pack_hw(woq), wovv=pack_hw(wov),
        bq=np.ascontiguousarray(
            (np.asarray(kbo, dtype=f32) @ Wq_).reshape(1, F)),
        bkfs=pack_b(bkf, BETA), bkfu=pack_b(bkf, 1.0),
        kb1s=pack_b(kb1, BETA), kb1u=pack_b(kb1, 1.0),
        kb2u=pack_b(kb2, 1.0),
    )
    return nsup, xts, mks, evs, weights, core_bounds


_CACHE = {}
LAST_RESULT = None


def kernel(x, E, num_batch, batch_seg, Wq, Wkf, bkf, Wvf, kW1, kb1, kW2, kb2,
           kWo, kbo, vW1, vW2, vWo, **_ignored):
    from concourse.bass_utils import run_bass_kernel_spmd

    nsup, xts, mks, evs, weights, core_bounds = _prep_host(
        x, E, batch_seg, Wq, Wkf, bkf, Wvf, kW1, kb1, kW2, kb2, kWo, kbo,
        vW1, vW2, vWo)

    if nsup not in _CACHE:
        _CACHE[nsup] = _build_program(nsup)
    nc = _CACHE[nsup]

    in_maps = [
        dict(weights, x=xts[c], mk=mks[c], ev=evs[c])
        for c in range(NCORES)
    ]
    res = run_bass_kernel_spmd(nc, in_maps, core_ids=list(range(NCORES)))
    global LAST_RESULT
    LAST_RESULT = res

    N = np.asarray(x).shape[0]
    out = np.empty((N, F), dtype=np.float32)
    for c in range(NCORES):
        n0, n1 = core_bounds[c], core_bounds[c + 1]
        out[n0:n1] = res.results[c]["out"][:, :n1 - n0].T.astype(np.float32)
    return out


# revision 6
# speedup vs baseline: 1.6419x; 1.0036x over previous
"""Trainium2 Bass kernel for NonlinearElectronicEmbedding (segment softmax).

Design ("T2", transposed / padding-free):
  - 512 molecules -> 64 consecutive molecules per core (8 cores). Atoms of
    a core's molecules form one contiguous run (batch_seg sorted); x is
    shipped TRANSPOSED (features on partitions, atoms on the free axis) in
    fp16, so there is no 128-atom padding at all.
  - Prelude computes the k/v tables from E via the ResidualMLPs in
    transposed layout (features on partitions), fusing Wq and kbo@Wq into
    the k-table:  dot(a) = x(a) . (k_mol @ Wq)[seg(a)].
  - Main loop over "supers" of 1024 atoms:
      dots  = kqT^T @ xT           (PE, all 64 molecules at once, fp16)
      e     = exp(dots/16)         (ACT, PSUM->SBUF fp16)
      S     = e * mask, partial = rowsum(S)   (DVE stt fused accum)
      anorm += partial; r = 1/(anorm+eps)     (tiny DVE)
      S[s-1] *= r  (per-partition scalar; every molecule of super s-1 is
                    closed by the end of super s since molecules < 1024)
      outT[s-1] = v16^T @ S[s-1]   (PE outer product, K=64)
      copy PSUM->SBUF fp16 (split ACT/DVE), DMA out.
  - mask is a host-built fp16 0/1 band matrix [64, NCpad] (bs sorted ->
    band). Garbage dot rows (wrong molecules) are zeroed by it; softmax
    shift invariance makes the seg_max pass unnecessary (args bounded).
  - Host does only layout work: transpose+fp16 cast in, transpose+fp32
    cast out.
HBM traffic/core ~ 26+6+26 MB (x + mask + out, fp16) -> memory roofline.
"""

import numpy as np

F = 256
B = 512
NCORES = 8
BC = B // NCORES  # molecules per core
P = 128
SUP = 1024        # atoms per super-group (2 PSUM banks of dots)
HB = SUP // 2     # 512, one PSUM bank
BETA = 1.702
EPS = 1e-8
INV_SQRT_F = 1.0 / 16.0


def _build_program(nsup):
    import concourse.bacc as bacc
    import concourse.mybir as mybir
    import concourse.tile as tile

    dt = mybir.dt
    f32 = dt.float32
    f16 = dt.float16
    AF = mybir.ActivationFunctionType
    ALU = mybir.AluOpType

    NCpad = nsup * SUP

    nc = bacc.Bacc(trn_type="TRN2")

    x_h = nc.dram_tensor("x", [2 * P, NCpad], f16, kind="ExternalInput")
    mk_h = nc.dram_tensor("mk", [BC, NCpad], f16, kind="ExternalInput")
    ev_h = nc.dram_tensor("ev", [1, BC], f32, kind="ExternalInput")
    wkf_h = nc.dram_tensor("wkf", [1, F], f32, kind="ExternalInput")
    wvf_h = nc.dram_tensor("wvf", [1, F], f32, kind="ExternalInput")
    kw1_h = nc.dram_tensor("kw1", [P, 2, 2, P], f32, kind="ExternalInput")
    kw2_h = nc.dram_tensor("kw2", [P, 2, 2, P], f32, kind="ExternalInput")
    vw1_h = nc.dram_tensor("vw1", [P, 2, 2, P], f32, kind="ExternalInput")
    vw2_h = nc.dram_tensor("vw2", [P, 2, 2, P], f32, kind="ExternalInput")
    woqk_h = nc.dram_tensor("woqk", [P, 2, 2, P], f32, kind="ExternalInput")
    wovv_h = nc.dram_tensor("wovv", [P, 2, 2, P], f32, kind="ExternalInput")
    bq_h = nc.dram_tensor("bq", [1, F], f32, kind="ExternalInput")
    # biases: [P, 2] chunked; *_s pre-multiplied by BETA, *_u raw
    bkfs_h = nc.dram_tensor("bkfs", [P, 2], f32, kind="ExternalInput")
    bkfu_h = nc.dram_tensor("bkfu", [P, 2], f32, kind="ExternalInput")
    kb1s_h = nc.dram_tensor("kb1s", [P, 2], f32, kind="ExternalInput")
    kb1u_h = nc.dram_tensor("kb1u", [P, 2], f32, kind="ExternalInput")
    kb2u_h = nc.dram_tensor("kb2u", [P, 2], f32, kind="ExternalInput")
    out_h = nc.dram_tensor("out", [2 * P, NCpad], f16, kind="ExternalOutput")

    # DRAM views with feature-chunk as a middle axis so one DMA moves both
    # 128-row chunks into/out of a [128, 2, SUP] SBUF tile
    xv = x_h[:].rearrange("(c p) j -> p c j", c=2)
    ov = out_h[:].rearrange("(c p) j -> p c j", c=2)

    with tile.TileContext(nc) as tc:
        with (
            tc.tile_pool(name="singles", bufs=1) as sg,
            tc.tile_pool(name="xpool", bufs=4) as xp,
            tc.tile_pool(name="mpool", bufs=4) as mp,
            tc.tile_pool(name="epool", bufs=2) as ep,
            tc.tile_pool(name="spool", bufs=4) as sp_,
            tc.tile_pool(name="opool", bufs=3) as op,
            tc.tile_pool(name="rpool", bufs=4) as rp,
        ):
            def load(name, h, shape):
                t_ = sg.tile(shape, f32, tag=name, name=name)
                nc.sync.dma_start(out=t_[:], in_=h[:])
                return t_

            ev_sb = load("ev", ev_h, [1, BC])
            wkf_sb = load("wkf", wkf_h, [1, F])
            wvf_sb = load("wvf", wvf_h, [1, F])
            kw1_sb = load("kw1", kw1_h, [P, 2, 2, P])
            kw2_sb = load("kw2", kw2_h, [P, 2, 2, P])
            vw1_sb = load("vw1", vw1_h, [P, 2, 2, P])
            vw2_sb = load("vw2", vw2_h, [P, 2, 2, P])
            woqk_sb = load("woqk", woqk_h, [P, 2, 2, P])
            wovv_sb = load("wovv", wovv_h, [P, 2, 2, P])
            bq_sb = load("bq", bq_h, [1, F])
            bkfs_sb = load("bkfs", bkfs_h, [P, 2])
            bkfu_sb = load("bkfu", bkfu_h, [P, 2])
            kb1s_sb = load("kb1s", kb1s_h, [P, 2])
            kb1u_sb = load("kb1u", kb1u_h, [P, 2])
            kb2u_sb = load("kb2u", kb2u_h, [P, 2])

            ones1 = sg.tile([1, BC], f32)
            nc.vector.memset(ones1[:], 1.0)

            kqT16 = sg.tile([P, 2, BC], f16)   # kqT16[f', c, b]
            v16 = sg.tile([BC, 2, P], f16)     # v16[b, c, f']
            anorm_run = sg.tile([BC, 1], f32)
            nc.vector.memset(anorm_run[:], 0.0)

            # ---- prelude: ResidualMLP in transposed layout ----
            # swish(y) = y * sigmoid(BETA*y); h_psum holds y - b.
            def swishT(c, h_psum, bs_ap, bu_ap, pre, keep_hb=False):
                sig = pre.tile([P, BC], f32, tag=f"sig_{c}", name="sig")
                nc.scalar.activation(sig[:], h_psum[:], AF.Sigmoid,
                                     bias=bs_ap if bs_ap is not None else 0.0,
                                     scale=BETA)
                if bu_ap is not None:
                    hb = pre.tile([P, BC], f32, tag=f"hb_{c}", name="hb")
                    nc.vector.tensor_scalar_add(hb[:], h_psum[:], bu_ap)
                elif keep_hb:
                    hb = pre.tile([P, BC], f32, tag=f"hb_{c}", name="hb")
                    nc.vector.tensor_copy(hb[:], h_psum[:])
                else:
                    hb = h_psum
                s = pre.tile([P, BC], f32, tag=f"s_{c}", name="s")
                nc.vector.tensor_mul(s[:], hb[:], sig[:])
                return (s, hb) if keep_hb else (s, None)

            def resmlp_T(wf_sb, b0s, b0u, w1_sb, b1s, b1u, w2_sb, b2u,
                         pre, ppre, branch):
                h0, s1, h1, s2, h2, s3, hb0 = [], [], [], [], [], [], []
                for c in (0, 1):
                    t_ = ppre.tile([P, BC], f32, tag=f"h0_{c}", name="h0")
                    nc.tensor.matmul(t_[:], wf_sb[0:1, c * P:(c + 1) * P],
                                     ev_sb[:], start=True, stop=True)
                    h0.append(t_)
                for c in (0, 1):
                    s, hb = swishT(
                        f"a{c}", h0[c],
                        b0s[:, c:c + 1] if b0s is not None else None,
                        b0u[:, c:c + 1] if b0u is not None else None,
                        pre, keep_hb=True)
                    s1.append(s)
                    hb0.append(hb if hb is not None else h0[c])
                for m in (0, 1):
                    t_ = ppre.tile([P, BC], f32, tag=f"h1_{m}", name="h1")
                    for k in (0, 1):
                        nc.tensor.matmul(t_[:], w1_sb[:, k, m, :], s1[k][:],
                                         start=(k == 0), stop=(k == 1))
                    h1.append(t_)
                for m in (0, 1):
                    s, _ = swishT(
                        f"b{m}", h1[m],
                        b1s[:, m:m + 1] if b1s is not None else None,
                        b1u[:, m:m + 1] if b1u is not None else None, pre)
                    s2.append(s)
                for m in (0, 1):
                    t_ = ppre.tile([P, BC], f32, tag=f"h2_{m}", name="h2")
                    for k in (0, 1):
                        nc.tensor.matmul(t_[:], w2_sb[:, k, m, :], s2[k][:],
                                         start=(k == 0), stop=(k == 1))
                    h2.append(t_)
                for m in (0, 1):
                    rt = pre.tile([P, BC], f32, tag=f"r_{m}_{branch}", name="rt")
                    nc.vector.tensor_add(rt[:], hb0[m][:], h2[m][:])
                    if b2u is not None:
                        nc.vector.tensor_scalar_add(rt[:], rt[:],
                                                    b2u[:, m:m + 1])
                    sig = pre.tile([P, BC], f32, tag=f"sig3_{m}", name="sig3")
                    nc.scalar.activation(sig[:], rt[:], AF.Sigmoid, bias=0.0,
                                         scale=BETA)
                    s = pre.tile([P, BC], f32, tag=f"s3_{m}_{branch}", name="s3")
                    nc.vector.tensor_mul(s[:], rt[:], sig[:])
                    s3.append(s)
                return s3

            with (
                tc.tile_pool(name="pre", bufs=2) as pre,
                tc.tile_pool(name="ppre", bufs=1, space="PSUM") as ppre,
                tc.tile_pool(name="ptab", bufs=1, space="PSUM") as ptab,
            ):
                s3k = resmlp_T(wkf_sb, bkfs_sb, bkfu_sb, kw1_sb, kb1s_sb,
                               kb1u_sb, kw2_sb, kb2u_sb, pre, ppre, "k")
                s3v = resmlp_T(wvf_sb, None, None, vw1_sb, None, None,
                               vw2_sb, None, pre, ppre, "v")
                # kqT[g, b] = sum_h s3k[h, b] * woq[h, g] + bq[g]
                pkq = ptab.tile([P, 2, BC], f32, tag="pkq")
                for c in (0, 1):
                    nc.tensor.matmul(pkq[:, c, :], woqk_sb[:, 0, c, :],
                                     s3k[0][:], start=True, stop=False)
                    nc.tensor.matmul(pkq[:, c, :], woqk_sb[:, 1, c, :],
                                     s3k[1][:], start=False, stop=False)
                    nc.tensor.matmul(pkq[:, c, :],
                                     bq_sb[0:1, c * P:(c + 1) * P],
                                     ones1[:], start=False, stop=True)
                nc.vector.tensor_copy(kqT16[:], pkq[:])
                # v16[b, f'] (chunked) = sum_h s3v[h, b] * wov[h, f']
                pv = ptab.tile([BC, 2, P], f32, tag="pv")
                for c in (0, 1):
                    for k in (0, 1):
                        nc.tensor.matmul(pv[:, c, :], s3v[k][:],
                                         wovv_sb[:, k, c, :],
                                         start=(k == 0), stop=(k == 1))
                nc.vector.tensor_copy(v16[:], pv[:])

            with (
                tc.tile_pool(name="pdot", bufs=2, space="PSUM") as pd_pool,
                tc.tile_pool(name="pout", bufs=2, space="PSUM") as po_pool,
            ):
                S_tiles = [None] * nsup
                r_tiles = [None] * nsup

                def pass2(s):
                    # S[s] *= r (all molecules of super s closed by now)
                    St = S_tiles[s]
                    rt = r_tiles[min(s + 1, nsup - 1)]
                    nc.vector.tensor_scalar_mul(St[:], St[:], rt[:])
                    out16 = op.tile([P, 2, SUP], f16, tag="out16")
                    for c in (0, 1):
                        po = po_pool.tile([P, 2, HB], f32, tag="po")
                        for b in (0, 1):
                            nc.tensor.matmul(
                                po[:, b, :], v16[:, c, :],
                                St[:, b * HB:(b + 1) * HB],
                                start=True, stop=True)
                        dst = out16[:, c, :].rearrange("p (b j) -> p b j", b=2)
                        # alternate copy engine to balance ACT/DVE load
                        if (s + c) % 2 == 0:
                            nc.scalar.activation(dst, po[:], AF.Copy)
                        else:
                            nc.vector.tensor_copy(dst, po[:])
                    nc.sync.dma_start(
                        out=ov[:, :, s * SUP:(s + 1) * SUP], in_=out16[:])

                x_tiles = [None] * nsup
                m_tiles = [None] * nsup

                def fetch(s):
                    cols = slice(s * SUP, (s + 1) * SUP)
                    x16 = xp.tile([P, 2, SUP], f16, tag="x16")
                    nc.sync.dma_start(out=x16[:], in_=xv[:, :, cols])
                    mk = mp.tile([BC, SUP], f16, tag="mk")
                    nc.sync.dma_start(out=mk[:], in_=mk_h[:, cols])
                    x_tiles[s], m_tiles[s] = x16, mk

                fetch(0)
                if nsup > 1:
                    fetch(1)
                for s in range(nsup):
                    if s + 2 < nsup:
                        fetch(s + 2)
                    if s >= 2:
                        pass2(s - 2)
                    x16, mk = x_tiles[s], m_tiles[s]

                    pd = pd_pool.tile([BC, 2, HB], f32, tag="pd")
                    for c in (0, 1):
                        for b in (0, 1):
                            nc.tensor.matmul(
                                pd[:, b, :], kqT16[:, c, :],
                                x16[:, c, b * HB:(b + 1) * HB],
                                start=(c == 0), stop=(c == 1),
                                skip_group_check=True)
                    e16 = ep.tile([BC, SUP], f16, tag="e16")
                    nc.scalar.activation(
                        e16[:].rearrange("p (b j) -> p b j", b=2), pd[:],
                        AF.Exp, bias=0.0, scale=INV_SQRT_F)
                    St = sp_.tile([BC, SUP], f16, tag="St")
                    part = rp.tile([BC, 1], f32, tag="part")
                    nc.vector.scalar_tensor_tensor(
                        St[:], e16[:], 1.0, mk[:], ALU.mult, ALU.mult,
                        accum_out=part[:])
                    S_tiles[s] = St
                    nc.vector.tensor_add(anorm_run[:], anorm_run[:], part[:])
                    rt = rp.tile([BC, 1], f32, tag="rt")
                    nc.vector.tensor_scalar_add(rt[:], anorm_run[:], EPS)
                    nc.vector.reciprocal(rt[:], rt[:])
                    r_tiles[s] = rt
                pass2(nsup - 2)
                pass2(nsup - 1)

    nc.compile()
    return nc


def _prep_host(x, E, batch_seg, Wq, Wkf, bkf, Wvf, kW1, kb1, kW2, kb2, kWo,
               kbo, vW1, vW2, vWo):
    f32 = np.float32
    f16 = np.float16
    bs = np.asarray(batch_seg).astype(np.int64)
    x = np.asarray(x, dtype=f32)
    N = x.shape[0]
    core_bounds = np.searchsorted(bs, np.arange(NCORES + 1) * BC, side="left")
    NCmax = int(np.max(np.diff(core_bounds)))
    nsup = max(1, -(-NCmax // SUP))
    NCpad = nsup * SUP

    xts, mks, evs = [], [], []
    E32 = np.asarray(E, dtype=f32)
    for c in range(NCORES):
        n0, n1 = core_bounds[c], core_bounds[c + 1]
        nc_ = n1 - n0
        xt = np.zeros((2 * P, NCpad), dtype=f16)
        xt[:, :nc_] = x[n0:n1].T.astype(f16)
        mk = np.zeros((BC, NCpad), dtype=f16)
        mk[:, :nc_] = (bs[n0:n1][None, :]
                       == (np.arange(BC) + c * BC)[:, None]).astype(f16)
        xts.append(xt)
        mks.append(mk)
        evs.append(np.ascontiguousarray(E32[c * BC:(c + 1) * BC].reshape(1, BC)))

    def pack_w(W):
        A = np.asarray(W, dtype=f32)
        return np.ascontiguousarray(A.reshape(2, P, 2, P).transpose(3, 2, 0, 1))

    def pack_hw(M):
        # M [F(h), F(g)] -> [P(h'), k(h-half), c(g-half), P(g')]
        return np.ascontiguousarray(
            M.reshape(2, P, 2, P).transpose(1, 0, 2, 3))

    def pack_b(v, scale):
        a = (np.asarray(v, dtype=f32) * f32(scale)).astype(f32)
        return np.ascontiguousarray(a.reshape(2, P).T)

    Wq_, kWo_, vWo_ = (np.asarray(a, dtype=f32) for a in (Wq, kWo, vWo))
    woq = (kWo_.T @ Wq_).astype(f32)   # [h, g]
    wov = vWo_.T.astype(f32)           # [h, f]
    weights = dict(
        wkf=np.ascontiguousarray(np.asarray(Wkf, dtype=f32).reshape(F)[None, :]),
        wvf=np.ascontiguousarray(np.asarray(Wvf, dtype=f32).reshape(F)[None, :]),
        kw1=pack_w(kW1), kw2=pack_w(kW2),
        vw1=pack_w(vW1), vw2=pack_w(vW2),
        woqk=pack_hw(woq), wovv=pack_hw(wov),
        bq=np.ascontiguousarray(
            (np.asarray(kbo, dtype=f32) @ Wq_).reshape(1, F)),
        bkfs=pack_b(bkf, BETA), bkfu=pack_b(bkf, 1.0),
        kb1s=pack_b(kb1, BETA), kb1u=pack_b(kb1, 1.0),
        kb2u=pack_b(kb2, 1.0),
    )
    return nsup, xts, mks, evs, weights, core_bounds


_CACHE = {}
LAST_RESULT = None


def kernel(x, E, num_batch, batch_seg, Wq, Wkf, bkf, Wvf, kW1, kb1, kW2, kb2,
           kWo, kbo, vW1, vW2, vWo, **_ignored):
    from concourse.bass_utils import run_bass_kernel_spmd

    nsup, xts, mks, evs, weights, core_bounds = _prep_host(
        x, E, batch_seg, Wq, Wkf, bkf, Wvf, kW1, kb1, kW2, kb2, kWo, kbo,
        vW1, vW2, vWo)

    if nsup not in _CACHE:
        _CACHE[nsup] = _build_program(nsup)
    nc = _CACHE[nsup]

    in_maps = [
        dict(weights, x=xts[c], mk=mks[c], ev=evs[c])
        for c in range(NCORES)
    ]
    res = run_bass_kernel_spmd(nc, in_maps, core_ids=list(range(NCORES)))
    global LAST_RESULT
    LAST_RESULT = res

    N = np.asarray(x).shape[0]
    out = np.empty((N, F), dtype=np.float32)
    for c in range(NCORES):
        n0, n1 = core_bounds[c], core_bounds[c + 1]
        out[n0:n1] = res.results[c]["out"][:, :n1 - n0].T.astype(np.float32)
    return out


# revision 11
# speedup vs baseline: 1.6567x; 1.0090x over previous
"""Trainium2 Bass kernel for NonlinearElectronicEmbedding (segment softmax).

Design ("T2", transposed / padding-free):
  - 512 molecules -> 64 consecutive molecules per core (8 cores). Atoms of
    a core's molecules form one contiguous run (batch_seg sorted); x is
    shipped TRANSPOSED (features on partitions, atoms on the free axis) in
    fp16, so there is no 128-atom padding at all.
  - Prelude computes the k/v tables from E via the ResidualMLPs in
    transposed layout (features on partitions), fusing Wq and kbo@Wq into
    the k-table:  dot(a) = x(a) . (k_mol @ Wq)[seg(a)].
  - Main loop over "supers" of 1024 atoms:
      dots  = kqT^T @ xT           (PE, all 64 molecules at once, fp16)
      e     = exp(dots/16)         (ACT, PSUM->SBUF fp16)
      S     = e * mask, partial = rowsum(S)   (DVE stt fused accum)
      anorm += partial; r = 1/(anorm+eps)     (tiny DVE)
      S[s-1] *= r  (per-partition scalar; every molecule of super s-1 is
                    closed by the end of super s since molecules < 1024)
      outT[s-1] = v16^T @ S[s-1]   (PE outer product, K=64)
      copy PSUM->SBUF fp16 (split ACT/DVE), DMA out.
  - mask is a host-built fp16 0/1 band matrix [64, NCpad] (bs sorted ->
    band). Garbage dot rows (wrong molecules) are zeroed by it; softmax
    shift invariance makes the seg_max pass unnecessary (args bounded).
  - Host does only layout work: transpose+fp16 cast in, transpose+fp32
    cast out.
HBM traffic/core ~ 26+6+26 MB (x + mask + out, fp16) -> memory roofline.
"""

import numpy as np

F = 256
B = 512
NCORES = 8
BC = B // NCORES  # molecules per core
P = 128
SUP = 1024        # atoms per super-group (2 PSUM banks of dots)
HB = SUP // 2     # 512, one PSUM bank
BETA = 1.702
EPS = 1e-8
INV_SQRT_F = 1.0 / 16.0


def _build_program(nsup):
    import concourse.bacc as bacc
    import concourse.mybir as mybir
    import concourse.tile as tile

    dt = mybir.dt
    f32 = dt.float32
    f16 = dt.bfloat16
    AF = mybir.ActivationFunctionType
    ALU = mybir.AluOpType

    NCpad = nsup * SUP

    nc = bacc.Bacc(trn_type="TRN2")

    x_h = nc.dram_tensor("x", [2 * P, NCpad], f16, kind="ExternalInput")
    mk_h = nc.dram_tensor("mk", [BC, NCpad], f16, kind="ExternalInput")
    ev_h = nc.dram_tensor("ev", [1, BC], f32, kind="ExternalInput")
    wkf_h = nc.dram_tensor("wkf", [1, F], f32, kind="ExternalInput")
    wvf_h = nc.dram_tensor("wvf", [1, F], f32, kind="ExternalInput")
    kw1_h = nc.dram_tensor("kw1", [P, 2, 2, P], f32, kind="ExternalInput")
    kw2_h = nc.dram_tensor("kw2", [P, 2, 2, P], f32, kind="ExternalInput")
    vw1_h = nc.dram_tensor("vw1", [P, 2, 2, P], f32, kind="ExternalInput")
    vw2_h = nc.dram_tensor("vw2", [P, 2, 2, P], f32, kind="ExternalInput")
    woqk_h = nc.dram_tensor("woqk", [P, 2, 2, P], f32, kind="ExternalInput")
    wovv_h = nc.dram_tensor("wovv", [P, 2, 2, P], f32, kind="ExternalInput")
    bq_h = nc.dram_tensor("bq", [1, F], f32, kind="ExternalInput")
    # biases: [P, 2] chunked; *_s pre-multiplied by BETA, *_u raw
    bkfs_h = nc.dram_tensor("bkfs", [P, 2], f32, kind="ExternalInput")
    bkfu_h = nc.dram_tensor("bkfu", [P, 2], f32, kind="ExternalInput")
    kb1s_h = nc.dram_tensor("kb1s", [P, 2], f32, kind="ExternalInput")
    kb1u_h = nc.dram_tensor("kb1u", [P, 2], f32, kind="ExternalInput")
    kb2u_h = nc.dram_tensor("kb2u", [P, 2], f32, kind="ExternalInput")
    out_h = nc.dram_tensor("out", [2 * P, NCpad], f16, kind="ExternalOutput")

    # DRAM views with feature-chunk as a middle axis so one DMA moves both
    # 128-row chunks into/out of a [128, 2, SUP] SBUF tile
    xv = x_h[:].rearrange("(c p) j -> p c j", c=2)
    ov = out_h[:].rearrange("(c p) j -> p c j", c=2)

    with tile.TileContext(nc) as tc:
        with (
            tc.tile_pool(name="singles", bufs=1) as sg,
            tc.tile_pool(name="xpool", bufs=4) as xp,
            tc.tile_pool(name="mpool", bufs=4) as mp,
            tc.tile_pool(name="epool", bufs=2) as ep,
            tc.tile_pool(name="spool", bufs=4) as sp_,
            tc.tile_pool(name="opool", bufs=3) as op,
            tc.tile_pool(name="rpool", bufs=4) as rp,
        ):
            def load(name, h, shape):
                t_ = sg.tile(shape, f32, tag=name, name=name)
                nc.sync.dma_start(out=t_[:], in_=h[:])
                return t_

            ev_sb = load("ev", ev_h, [1, BC])
            wkf_sb = load("wkf", wkf_h, [1, F])
            wvf_sb = load("wvf", wvf_h, [1, F])
            kw1_sb = load("kw1", kw1_h, [P, 2, 2, P])
            kw2_sb = load("kw2", kw2_h, [P, 2, 2, P])
            vw1_sb = load("vw1", vw1_h, [P, 2, 2, P])
            vw2_sb = load("vw2", vw2_h, [P, 2, 2, P])
            woqk_sb = load("woqk", woqk_h, [P, 2, 2, P])
            wovv_sb = load("wovv", wovv_h, [P, 2, 2, P])
            bq_sb = load("bq", bq_h, [1, F])
            bkfs_sb = load("bkfs", bkfs_h, [P, 2])
            bkfu_sb = load("bkfu", bkfu_h, [P, 2])
            kb1s_sb = load("kb1s", kb1s_h, [P, 2])
            kb1u_sb = load("kb1u", kb1u_h, [P, 2])
            kb2u_sb = load("kb2u", kb2u_h, [P, 2])

            ones1 = sg.tile([1, BC], f32)
            nc.vector.memset(ones1[:], 1.0)

            kqT16 = sg.tile([P, 2, BC], f16)   # kqT16[f', c, b]
            v16 = sg.tile([BC, 2, P], f16)     # v16[b, c, f']
            anorm_run = sg.tile([BC, 1], f32)
            nc.vector.memset(anorm_run[:], 0.0)

            # ---- prelude: ResidualMLP in transposed layout ----
            # swish(y) = y * sigmoid(BETA*y); h_psum holds y - b.
            def swishT(c, h_psum, bs_ap, bu_ap, pre, keep_hb=False):
                sig = pre.tile([P, BC], f32, tag=f"sig_{c}", name="sig")
                nc.scalar.activation(sig[:], h_psum[:], AF.Sigmoid,
                                     bias=bs_ap if bs_ap is not None else 0.0,
                                     scale=BETA)
                if bu_ap is not None:
                    hb = pre.tile([P, BC], f32, tag=f"hb_{c}", name="hb")
                    nc.vector.tensor_scalar_add(hb[:], h_psum[:], bu_ap)
                elif keep_hb:
                    hb = pre.tile([P, BC], f32, tag=f"hb_{c}", name="hb")
                    nc.vector.tensor_copy(hb[:], h_psum[:])
                else:
                    hb = h_psum
                s = pre.tile([P, BC], f32, tag=f"s_{c}", name="s")
                nc.vector.tensor_mul(s[:], hb[:], sig[:])
                return (s, hb) if keep_hb else (s, None)

            def resmlp_T(wf_sb, b0s, b0u, w1_sb, b1s, b1u, w2_sb, b2u,
                         pre, ppre, branch):
                h0, s1, h1, s2, h2, s3, hb0 = [], [], [], [], [], [], []
                for c in (0, 1):
                    t_ = ppre.tile([P, BC], f32, tag=f"h0_{c}", name="h0")
                    nc.tensor.matmul(t_[:], wf_sb[0:1, c * P:(c + 1) * P],
                                     ev_sb[:], start=True, stop=True)
                    h0.append(t_)
                for c in (0, 1):
                    s, hb = swishT(
                        f"a{c}", h0[c],
                        b0s[:, c:c + 1] if b0s is not None else None,
                        b0u[:, c:c + 1] if b0u is not None else None,
                        pre, keep_hb=True)
                    s1.append(s)
                    hb0.append(hb if hb is not None else h0[c])
                for m in (0, 1):
                    t_ = ppre.tile([P, BC], f32, tag=f"h1_{m}", name="h1")
                    for k in (0, 1):
                        nc.tensor.matmul(t_[:], w1_sb[:, k, m, :], s1[k][:],
                                         start=(k == 0), stop=(k == 1))
                    h1.append(t_)
                for m in (0, 1):
                    s, _ = swishT(
                        f"b{m}", h1[m],
                        b1s[:, m:m + 1] if b1s is not None else None,
                        b1u[:, m:m + 1] if b1u is not None else None, pre)
                    s2.append(s)
                for m in (0, 1):
                    t_ = ppre.tile([P, BC], f32, tag=f"h2_{m}", name="h2")
                    for k in (0, 1):
                        nc.tensor.matmul(t_[:], w2_sb[:, k, m, :], s2[k][:],
                                         start=(k == 0), stop=(k == 1))
                    h2.append(t_)
                for m in (0, 1):
                    rt = pre.tile([P, BC], f32, tag=f"r_{m}_{branch}", name="rt")
                    nc.vector.tensor_add(rt[:], hb0[m][:], h2[m][:])
                    if b2u is not None:
                        nc.vector.tensor_scalar_add(rt[:], rt[:],
                                                    b2u[:, m:m + 1])
                    sig = pre.tile([P, BC], f32, tag=f"sig3_{m}", name="sig3")
                    nc.scalar.activation(sig[:], rt[:], AF.Sigmoid, bias=0.0,
                                         scale=BETA)
                    s = pre.tile([P, BC], f32, tag=f"s3_{m}_{branch}", name="s3")
                    nc.vector.tensor_mul(s[:], rt[:], sig[:])
                    s3.append(s)
                return s3

            with (
                tc.tile_pool(name="pre", bufs=2) as pre,
                tc.tile_pool(name="ppre", bufs=1, space="PSUM") as ppre,
                tc.tile_pool(name="ptab", bufs=1, space="PSUM") as ptab,
            ):
                s3k = resmlp_T(wkf_sb, bkfs_sb, bkfu_sb, kw1_sb, kb1s_sb,
                               kb1u_sb, kw2_sb, kb2u_sb, pre, ppre, "k")
                s3v = resmlp_T(wvf_sb, None, None, vw1_sb, None, None,
                               vw2_sb, None, pre, ppre, "v")
                # kqT[g, b] = sum_h s3k[h, b] * woq[h, g] + bq[g]
                pkq = ptab.tile([P, 2, BC], f32, tag="pkq")
                for c in (0, 1):
                    nc.tensor.matmul(pkq[:, c, :], woqk_sb[:, 0, c, :],
                                     s3k[0][:], start=True, stop=False)
                    nc.tensor.matmul(pkq[:, c, :], woqk_sb[:, 1, c, :],
                                     s3k[1][:], start=False, stop=False)
                    nc.tensor.matmul(pkq[:, c, :],
                                     bq_sb[0:1, c * P:(c + 1) * P],
                                     ones1[:], start=False, stop=True)
                nc.vector.tensor_copy(kqT16[:], pkq[:])
                # v16[b, f'] (chunked) = sum_h s3v[h, b] * wov[h, f']
                pv = ptab.tile([BC, 2, P], f32, tag="pv")
                for c in (0, 1):
                    for k in (0, 1):
                        nc.tensor.matmul(pv[:, c, :], s3v[k][:],
                                         wovv_sb[:, k, c, :],
                                         start=(k == 0), stop=(k == 1))
                nc.vector.tensor_copy(v16[:], pv[:])

            with (
                tc.tile_pool(name="pdot", bufs=2, space="PSUM") as pd_pool,
                tc.tile_pool(name="pout", bufs=2, space="PSUM") as po_pool,
            ):
                S_tiles = [None] * nsup
                r_tiles = [None] * nsup

                def pass2(s):
                    # S[s] *= r (all molecules of super s closed by now)
                    St = S_tiles[s]
                    rt = r_tiles[min(s + 1, nsup - 1)]
                    nc.vector.tensor_scalar_mul(St[:], St[:], rt[:])
                    out16 = op.tile([P, 2, SUP], f16, tag="out16")
                    for c in (0, 1):
                        po = po_pool.tile([P, 2, HB], f32, tag="po")
                        for b in (0, 1):
                            nc.tensor.matmul(
                                po[:, b, :], v16[:, c, :],
                                St[:, b * HB:(b + 1) * HB],
                                start=True, stop=True)
                        dst = out16[:, c, :].rearrange("p (b j) -> p b j", b=2)
                        # alternate copy engine to balance ACT/DVE load
                        if (s + c) % 2 == 0:
                            nc.scalar.activation(dst, po[:], AF.Copy)
                        else:
                            nc.vector.tensor_copy(dst, po[:])
                    nc.sync.dma_start(
                        out=ov[:, :, s * SUP:(s + 1) * SUP], in_=out16[:])

                x_tiles = [None] * nsup
                m_tiles = [None] * nsup

                def fetch(s):
                    cols = slice(s * SUP, (s + 1) * SUP)
                    x16 = xp.tile([P, 2, SUP], f16, tag="x16")
                    nc.sync.dma_start(out=x16[:], in_=xv[:, :, cols])
                    mk = mp.tile([BC, SUP], f16, tag="mk")
                    nc.sync.dma_start(out=mk[:], in_=mk_h[:, cols])
                    x_tiles[s], m_tiles[s] = x16, mk

                fetch(0)
                if nsup > 1:
                    fetch(1)
                for s in range(nsup):
                    if s + 2 < nsup:
                        fetch(s + 2)
                    if s >= 2:
                        pass2(s - 2)
                    x16, mk = x_tiles[s], m_tiles[s]

                    pd = pd_pool.tile([BC, 2, HB], f32, tag="pd")
                    for c in (0, 1):
                        for b in (0, 1):
                            nc.tensor.matmul(
                                pd[:, b, :], kqT16[:, c, :],
                                x16[:, c, b * HB:(b + 1) * HB],
                                start=(c == 0), stop=(c == 1),
                                skip_group_check=True)
                    e16 = ep.tile([BC, SUP], f16, tag="e16")
                    nc.scalar.activation(
                        e16[:].rearrange("p (b j) -> p b j", b=2), pd[:],
                        AF.Exp, bias=0.0, scale=INV_SQRT_F)
                    St = sp_.tile([BC, SUP], f16, tag="St")
                    part = rp.tile([BC, 1], f32, tag="part")
                    nc.vector.scalar_tensor_tensor(
                        St[:], e16[:], 1.0, mk[:], ALU.mult, ALU.mult,
                        accum_out=part[:])
                    S_tiles[s] = St
                    nc.vector.tensor_add(anorm_run[:], anorm_run[:], part[:])
                    rt = rp.tile([BC, 1], f32, tag="rt")
                    nc.vector.tensor_scalar_add(rt[:], anorm_run[:], EPS)
                    nc.vector.reciprocal(rt[:], rt[:])
                    r_tiles[s] = rt
                pass2(nsup - 2)
                pass2(nsup - 1)

    nc.compile()
    return nc


def _prep_host(x, E, batch_seg, Wq, Wkf, bkf, Wvf, kW1, kb1, kW2, kb2, kWo,
               kbo, vW1, vW2, vWo):
    f32 = np.float32
    import ml_dtypes
    f16 = ml_dtypes.bfloat16
    bs = np.asarray(batch_seg).astype(np.int64)
    x = np.asarray(x, dtype=f32)
    N = x.shape[0]
    core_bounds = np.searchsorted(bs, np.arange(NCORES + 1) * BC, side="left")
    NCmax = int(np.max(np.diff(core_bounds)))
    nsup = max(1, -(-NCmax // SUP))
    NCpad = nsup * SUP

    xts, mks, evs = [], [], []
    E32 = np.asarray(E, dtype=f32)
    for c in range(NCORES):
        n0, n1 = core_bounds[c], core_bounds[c + 1]
        nc_ = n1 - n0
        xt = np.zeros((2 * P, NCpad), dtype=f16)
        xt[:, :nc_] = x[n0:n1].T.astype(f16)
        mk = np.zeros((BC, NCpad), dtype=f16)
        mk[:, :nc_] = (bs[n0:n1][None, :]
                       == (np.arange(BC) + c * BC)[:, None]).astype(f16)
        xts.append(xt)
        mks.append(mk)
        evs.append(np.ascontiguousarray(E32[c * BC:(c + 1) * BC].reshape(1, BC)))

    def pack_w(W):
        A = np.asarray(W, dtype=f32)
        return np.ascontiguousarray(A.reshape(2, P, 2, P).transpose(3, 2, 0, 1))

    def pack_hw(M):
        # M [F(h), F(g)] -> [P(h'), k(h-half), c(g-half), P(g')]
        return np.ascontiguousarray(
            M.reshape(2, P, 2, P).transpose(1, 0, 2, 3))

    def pack_b(v, scale):
        a = (np.asarray(v, dtype=f32) * f32(scale)).astype(f32)
        return np.ascontiguousarray(a.reshape(2, P).T)

    Wq_, kWo_, vWo_ = (np.asarray(a, dtype=f32) for a in (Wq, kWo, vWo))
    woq = (kWo_.T @ Wq_).astype(f32)   # [h, g]
    wov = vWo_.T.astype(f32)           # [h, f]
    weights = dict(
        wkf=np.ascontiguousarray(np.asarray(Wkf, dtype=f32).reshape(F)[None, :]),
        wvf=np.ascontiguousarray(np.asarray(Wvf, dtype=f32).reshape(F)[None, :]),
        kw1=pack_w(kW1), kw2=pack_w(kW2),
        vw1=pack_w(vW1), vw2=pack_w(vW2),
        woqk=pack_hw(woq), wovv=pack_hw(wov),
        bq=np.ascontiguousarray(
            (np.asarray(kbo, dtype=f32) @ Wq_).reshape(1, F)),
        bkfs=pack_b(bkf, BETA), bkfu=pack_b(bkf, 1.0),
        kb1s=pack_b(kb1, BETA), kb1u=pack_b(kb1, 1.0),
        kb2u=pack_b(kb2, 1.0),
    )
    return nsup, xts, mks, evs, weights, core_bounds


_CACHE = {}
LAST_RESULT = None


def kernel(x, E, num_batch, batch_seg, Wq, Wkf, bkf, Wvf, kW1, kb1, kW2, kb2,
           kWo, kbo, vW1, vW2, vWo, **_ignored):
    from concourse.bass_utils import run_bass_kernel_spmd

    nsup, xts, mks, evs, weights, core_bounds = _prep_host(
        x, E, batch_seg, Wq, Wkf, bkf, Wvf, kW1, kb1, kW2, kb2, kWo, kbo,
        vW1, vW2, vWo)

    if nsup not in _CACHE:
        _CACHE[nsup] = _build_program(nsup)
    nc = _CACHE[nsup]

    in_maps = [
        dict(weights, x=xts[c], mk=mks[c], ev=evs[c])
        for c in range(NCORES)
    ]
    res = run_bass_kernel_spmd(nc, in_maps, core_ids=list(range(NCORES)))
    global LAST_RESULT
    LAST_RESULT = res

    N = np.asarray(x).shape[0]
    out = np.empty((N, F), dtype=np.float32)
    for c in range(NCORES):
        n0, n1 = core_bounds[c], core_bounds[c + 1]
        out[n0:n1] = res.results[c]["out"][:, :n1 - n0].T.astype(np.float32)
    return out


# revision 12
# speedup vs baseline: 1.9191x; 1.1584x over previous
"""Trainium2 Bass kernel for NonlinearElectronicEmbedding (segment softmax).

Design ("T2", transposed / padding-free):
  - 512 molecules -> 64 consecutive molecules per core (8 cores). Atoms of
    a core's molecules form one contiguous run (batch_seg sorted); x is
    shipped TRANSPOSED (features on partitions, atoms on the free axis) in
    fp16, so there is no 128-atom padding at all.
  - Prelude computes the k/v tables from E via the ResidualMLPs in
    transposed layout (features on partitions), fusing Wq and kbo@Wq into
    the k-table:  dot(a) = x(a) . (k_mol @ Wq)[seg(a)].
  - Main loop over "supers" of 1024 atoms:
      dots  = kqT^T @ xT           (PE, all 64 molecules at once, fp16)
      e     = exp(dots/16)         (ACT, PSUM->SBUF fp16)
      S     = e * mask, partial = rowsum(S)   (DVE stt fused accum)
      anorm += partial; r = 1/(anorm+eps)     (tiny DVE)
      S[s-1] *= r  (per-partition scalar; every molecule of super s-1 is
                    closed by the end of super s since molecules < 1024)
      outT[s-1] = v16^T @ S[s-1]   (PE outer product, K=64)
      copy PSUM->SBUF fp16 (split ACT/DVE), DMA out.
  - mask is a host-built fp16 0/1 band matrix [64, NCpad] (bs sorted ->
    band). Garbage dot rows (wrong molecules) are zeroed by it; softmax
    shift invariance makes the seg_max pass unnecessary (args bounded).
  - Host does only layout work: transpose+fp16 cast in, transpose+fp32
    cast out.
HBM traffic/core ~ 26+6+26 MB (x + mask + out, fp16) -> memory roofline.
"""

import numpy as np

F = 256
B = 512
NCORES = 8
BC = B // NCORES  # molecules per core
P = 128
SUP = 1024        # atoms per super-group (2 PSUM banks of dots)
HB = SUP // 2     # 512, one PSUM bank
BETA = 1.702
EPS = 1e-8
INV_SQRT_F = 1.0 / 16.0


def _build_program(nsup):
    import concourse.bacc as bacc
    import concourse.mybir as mybir
    import concourse.tile as tile

    dt = mybir.dt
    f32 = dt.float32
    f16 = dt.bfloat16
    AF = mybir.ActivationFunctionType
    ALU = mybir.AluOpType

    NCpad = nsup * SUP

    nc = bacc.Bacc(trn_type="TRN2")

    x_h = nc.dram_tensor("x", [2 * P, NCpad], f16, kind="ExternalInput")
    mk_h = nc.dram_tensor("mk", [BC, NCpad], f16, kind="ExternalInput")
    ev_h = nc.dram_tensor("ev", [1, BC], f32, kind="ExternalInput")
    wkf_h = nc.dram_tensor("wkf", [1, F], f32, kind="ExternalInput")
    wvf_h = nc.dram_tensor("wvf", [1, F], f32, kind="ExternalInput")
    kw1_h = nc.dram_tensor("kw1", [P, 2, 2, P], f32, kind="ExternalInput")
    kw2_h = nc.dram_tensor("kw2", [P, 2, 2, P], f32, kind="ExternalInput")
    vw1_h = nc.dram_tensor("vw1", [P, 2, 2, P], f32, kind="ExternalInput")
    vw2_h = nc.dram_tensor("vw2", [P, 2, 2, P], f32, kind="ExternalInput")
    woqk_h = nc.dram_tensor("woqk", [P, 2, 2, P], f32, kind="ExternalInput")
    wovv_h = nc.dram_tensor("wovv", [P, 2, 2, P], f32, kind="ExternalInput")
    bq_h = nc.dram_tensor("bq", [1, F], f32, kind="ExternalInput")
    # biases: [P, 2] chunked; *_s pre-multiplied by BETA, *_u raw
    bkfs_h = nc.dram_tensor("bkfs", [P, 2], f32, kind="ExternalInput")
    bkfu_h = nc.dram_tensor("bkfu", [P, 2], f32, kind="ExternalInput")
    kb1s_h = nc.dram_tensor("kb1s", [P, 2], f32, kind="ExternalInput")
    kb1u_h = nc.dram_tensor("kb1u", [P, 2], f32, kind="ExternalInput")
    kb2u_h = nc.dram_tensor("kb2u", [P, 2], f32, kind="ExternalInput")
    out_h = nc.dram_tensor("out", [2 * P, NCpad], f16, kind="ExternalOutput")

    # DRAM views with feature-chunk as a middle axis so one DMA moves both
    # 128-row chunks into/out of a [128, 2, SUP] SBUF tile
    xv = x_h[:].rearrange("(c p) j -> p c j", c=2)
    ov = out_h[:].rearrange("(c p) j -> p c j", c=2)

    with tile.TileContext(nc) as tc:
        with (
            tc.tile_pool(name="singles", bufs=1) as sg,
            tc.tile_pool(name="xpool", bufs=4) as xp,
            tc.tile_pool(name="mpool", bufs=4) as mp,
            tc.tile_pool(name="epool", bufs=2) as ep,
            tc.tile_pool(name="spool", bufs=5) as sp_,
            tc.tile_pool(name="opool", bufs=3) as op,
            tc.tile_pool(name="rpool", bufs=5) as rp,
        ):
            def load(name, h, shape):
                t_ = sg.tile(shape, f32, tag=name, name=name)
                nc.sync.dma_start(out=t_[:], in_=h[:])
                return t_

            ev_sb = load("ev", ev_h, [1, BC])
            wkf_sb = load("wkf", wkf_h, [1, F])
            wvf_sb = load("wvf", wvf_h, [1, F])
            kw1_sb = load("kw1", kw1_h, [P, 2, 2, P])
            kw2_sb = load("kw2", kw2_h, [P, 2, 2, P])
            vw1_sb = load("vw1", vw1_h, [P, 2, 2, P])
            vw2_sb = load("vw2", vw2_h, [P, 2, 2, P])
            woqk_sb = load("woqk", woqk_h, [P, 2, 2, P])
            wovv_sb = load("wovv", wovv_h, [P, 2, 2, P])
            bq_sb = load("bq", bq_h, [1, F])
            bkfs_sb = load("bkfs", bkfs_h, [P, 2])
            bkfu_sb = load("bkfu", bkfu_h, [P, 2])
            kb1s_sb = load("kb1s", kb1s_h, [P, 2])
            kb1u_sb = load("kb1u", kb1u_h, [P, 2])
            kb2u_sb = load("kb2u", kb2u_h, [P, 2])

            ones1 = sg.tile([1, BC], f32)
            nc.vector.memset(ones1[:], 1.0)

            kqT16 = sg.tile([P, 2, BC], f16)   # kqT16[f', c, b]
            v16 = sg.tile([BC, 2, P], f16)     # v16[b, c, f']
            anorm_run = sg.tile([BC, 1], f32)
            nc.vector.memset(anorm_run[:], 0.0)

            # ---- prelude: ResidualMLP in transposed layout ----
            # swish(y) = y * sigmoid(BETA*y); h_psum holds y - b.
            def swishT(c, h_psum, bs_ap, bu_ap, pre, keep_hb=False):
                sig = pre.tile([P, BC], f32, tag=f"sig_{c}", name="sig")
                nc.scalar.activation(sig[:], h_psum[:], AF.Sigmoid,
                                     bias=bs_ap if bs_ap is not None else 0.0,
                                     scale=BETA)
                if bu_ap is not None:
                    hb = pre.tile([P, BC], f32, tag=f"hb_{c}", name="hb")
                    nc.vector.tensor_scalar_add(hb[:], h_psum[:], bu_ap)
                elif keep_hb:
                    hb = pre.tile([P, BC], f32, tag=f"hb_{c}", name="hb")
                    nc.vector.tensor_copy(hb[:], h_psum[:])
                else:
                    hb = h_psum
                s = pre.tile([P, BC], f32, tag=f"s_{c}", name="s")
                nc.vector.tensor_mul(s[:], hb[:], sig[:])
                return (s, hb) if keep_hb else (s, None)

            def resmlp_T(wf_sb, b0s, b0u, w1_sb, b1s, b1u, w2_sb, b2u,
                         pre, ppre, branch):
                h0, s1, h1, s2, h2, s3, hb0 = [], [], [], [], [], [], []
                for c in (0, 1):
                    t_ = ppre.tile([P, BC], f32, tag=f"h0_{c}", name="h0")
                    nc.tensor.matmul(t_[:], wf_sb[0:1, c * P:(c + 1) * P],
                                     ev_sb[:], start=True, stop=True)
                    h0.append(t_)
                for c in (0, 1):
                    s, hb = swishT(
                        f"a{c}", h0[c],
                        b0s[:, c:c + 1] if b0s is not None else None,
                        b0u[:, c:c + 1] if b0u is not None else None,
                        pre, keep_hb=True)
                    s1.append(s)
                    hb0.append(hb if hb is not None else h0[c])
                for m in (0, 1):
                    t_ = ppre.tile([P, BC], f32, tag=f"h1_{m}", name="h1")
                    for k in (0, 1):
                        nc.tensor.matmul(t_[:], w1_sb[:, k, m, :], s1[k][:],
                                         start=(k == 0), stop=(k == 1))
                    h1.append(t_)
                for m in (0, 1):
                    s, _ = swishT(
                        f"b{m}", h1[m],
                        b1s[:, m:m + 1] if b1s is not None else None,
                        b1u[:, m:m + 1] if b1u is not None else None, pre)
                    s2.append(s)
                for m in (0, 1):
                    t_ = ppre.tile([P, BC], f32, tag=f"h2_{m}", name="h2")
                    for k in (0, 1):
                        nc.tensor.matmul(t_[:], w2_sb[:, k, m, :], s2[k][:],
                                         start=(k == 0), stop=(k == 1))
                    h2.append(t_)
                for m in (0, 1):
                    rt = pre.tile([P, BC], f32, tag=f"r_{m}_{branch}", name="rt")
                    nc.vector.tensor_add(rt[:], hb0[m][:], h2[m][:])
                    if b2u is not None:
                        nc.vector.tensor_scalar_add(rt[:], rt[:],
                                                    b2u[:, m:m + 1])
                    sig = pre.tile([P, BC], f32, tag=f"sig3_{m}", name="sig3")
                    nc.scalar.activation(sig[:], rt[:], AF.Sigmoid, bias=0.0,
                                         scale=BETA)
                    s = pre.tile([P, BC], f32, tag=f"s3_{m}_{branch}", name="s3")
                    nc.vector.tensor_mul(s[:], rt[:], sig[:])
                    s3.append(s)
                return s3

            with (
                tc.tile_pool(name="pre", bufs=2) as pre,
                tc.tile_pool(name="ppre", bufs=1, space="PSUM") as ppre,
                tc.tile_pool(name="ptab", bufs=1, space="PSUM") as ptab,
            ):
                s3k = resmlp_T(wkf_sb, bkfs_sb, bkfu_sb, kw1_sb, kb1s_sb,
                               kb1u_sb, kw2_sb, kb2u_sb, pre, ppre, "k")
                s3v = resmlp_T(wvf_sb, None, None, vw1_sb, None, None,
                               vw2_sb, None, pre, ppre, "v")
                # kqT[g, b] = sum_h s3k[h, b] * woq[h, g] + bq[g]
                pkq = ptab.tile([P, 2, BC], f32, tag="pkq")
                for c in (0, 1):
                    nc.tensor.matmul(pkq[:, c, :], woqk_sb[:, 0, c, :],
                                     s3k[0][:], start=True, stop=False)
                    nc.tensor.matmul(pkq[:, c, :], woqk_sb[:, 1, c, :],
                                     s3k[1][:], start=False, stop=False)
                    nc.tensor.matmul(pkq[:, c, :],
                                     bq_sb[0:1, c * P:(c + 1) * P],
                                     ones1[:], start=False, stop=True)
                nc.vector.tensor_copy(kqT16[:], pkq[:])
                # v16[b, f'] (chunked) = sum_h s3v[h, b] * wov[h, f']
                pv = ptab.tile([BC, 2, P], f32, tag="pv")
                for c in (0, 1):
                    for k in (0, 1):
                        nc.tensor.matmul(pv[:, c, :], s3v[k][:],
                                         wovv_sb[:, k, c, :],
                                         start=(k == 0), stop=(k == 1))
                nc.vector.tensor_copy(v16[:], pv[:])

            with (
                tc.tile_pool(name="pdot", bufs=2, space="PSUM") as pd_pool,
                tc.tile_pool(name="pout", bufs=2, space="PSUM") as po_pool,
            ):
                S_tiles = [None] * nsup
                r_tiles = [None] * nsup

                def pass2(s):
                    # S[s] *= r (all molecules of super s closed by now)
                    St = S_tiles[s]
                    rt = r_tiles[min(s + 1, nsup - 1)]
                    nc.vector.tensor_scalar_mul(St[:], St[:], rt[:])
                    out16 = op.tile([P, 2, SUP], f16, tag="out16")
                    for c in (0, 1):
                        po = po_pool.tile([P, 2, HB], f32, tag="po")
                        for b in (0, 1):
                            nc.tensor.matmul(
                                po[:, b, :], v16[:, c, :],
                                St[:, b * HB:(b + 1) * HB],
                                start=True, stop=True)
                        dst = out16[:, c, :].rearrange("p (b j) -> p b j", b=2)
                        nc.scalar.activation(dst, po[:], AF.Copy)
                    nc.sync.dma_start(
                        out=ov[:, :, s * SUP:(s + 1) * SUP], in_=out16[:])

                x_tiles = [None] * nsup
                m_tiles = [None] * nsup

                def fetch(s):
                    cols = slice(s * SUP, (s + 1) * SUP)
                    x16 = xp.tile([P, 2, SUP], f16, tag="x16")
                    nc.sync.dma_start(out=x16[:], in_=xv[:, :, cols])
                    mk = mp.tile([BC, SUP], f16, tag="mk")
                    nc.sync.dma_start(out=mk[:], in_=mk_h[:, cols])
                    x_tiles[s], m_tiles[s] = x16, mk

                fetch(0)
                if nsup > 1:
                    fetch(1)
                for s in range(nsup):
                    if s + 2 < nsup:
                        fetch(s + 2)
                    if s >= 3:
                        pass2(s - 3)
                    x16, mk = x_tiles[s], m_tiles[s]

                    pd = pd_pool.tile([BC, 2, HB], f32, tag="pd")
                    for c in (0, 1):
                        for b in (0, 1):
                            nc.tensor.matmul(
                                pd[:, b, :], kqT16[:, c, :],
                                x16[:, c, b * HB:(b + 1) * HB],
                                start=(c == 0), stop=(c == 1),
                                skip_group_check=True)
                    e16 = ep.tile([BC, SUP], f16, tag="e16")
                    nc.scalar.activation(
                        e16[:].rearrange("p (b j) -> p b j", b=2), pd[:],
                        AF.Exp, bias=0.0, scale=INV_SQRT_F)
                    St = sp_.tile([BC, SUP], f16, tag="St")
                    part = rp.tile([BC, 1], f32, tag="part")
                    nc.vector.scalar_tensor_tensor(
                        St[:], e16[:], 1.0, mk[:], ALU.mult, ALU.mult,
                        accum_out=part[:])
                    S_tiles[s] = St
                    nc.vector.tensor_add(anorm_run[:], anorm_run[:], part[:])
                    rt = rp.tile([BC, 1], f32, tag="rt")
                    nc.vector.tensor_scalar_add(rt[:], anorm_run[:], EPS)
                    nc.vector.reciprocal(rt[:], rt[:])
                    r_tiles[s] = rt
                pass2(nsup - 3)
                pass2(nsup - 2)
                pass2(nsup - 1)

    nc.compile()
    return nc


def _prep_host(x, E, batch_seg, Wq, Wkf, bkf, Wvf, kW1, kb1, kW2, kb2, kWo,
               kbo, vW1, vW2, vWo):
    f32 = np.float32
    import ml_dtypes
    f16 = ml_dtypes.bfloat16
    bs = np.asarray(batch_seg).astype(np.int64)
    x = np.asarray(x, dtype=f32)
    N = x.shape[0]
    core_bounds = np.searchsorted(bs, np.arange(NCORES + 1) * BC, side="left")
    NCmax = int(np.max(np.diff(core_bounds)))
    nsup = max(1, -(-NCmax // SUP))
    NCpad = nsup * SUP

    xts, mks, evs = [], [], []
    E32 = np.asarray(E, dtype=f32)
    for c in range(NCORES):
        n0, n1 = core_bounds[c], core_bounds[c + 1]
        nc_ = n1 - n0
        xt = np.zeros((2 * P, NCpad), dtype=f16)
        xt[:, :nc_] = x[n0:n1].T.astype(f16)
        mk = np.zeros((BC, NCpad), dtype=f16)
        mk[:, :nc_] = (bs[n0:n1][None, :]
                       == (np.arange(BC) + c * BC)[:, None]).astype(f16)
        xts.append(xt)
        mks.append(mk)
        evs.append(np.ascontiguousarray(E32[c * BC:(c + 1) * BC].reshape(1, BC)))

    def pack_w(W):
        A = np.asarray(W, dtype=f32)
        return np.ascontiguousarray(A.reshape(2, P, 2, P).transpose(3, 2, 0, 1))

    def pack_hw(M):
        # M [F(h), F(g)] -> [P(h'), k(h-half), c(g-half), P(g')]
        return np.ascontiguousarray(
            M.reshape(2, P, 2, P).transpose(1, 0, 2, 3))

    def pack_b(v, scale):
        a = (np.asarray(v, dtype=f32) * f32(scale)).astype(f32)
        return np.ascontiguousarray(a.reshape(2, P).T)

    Wq_, kWo_, vWo_ = (np.asarray(a, dtype=f32) for a in (Wq, kWo, vWo))
    woq = (kWo_.T @ Wq_).astype(f32)   # [h, g]
    wov = vWo_.T.astype(f32)           # [h, f]
    weights = dict(
        wkf=np.ascontiguousarray(np.asarray(Wkf, dtype=f32).reshape(F)[None, :]),
        wvf=np.ascontiguousarray(np.asarray(Wvf, dtype=f32).reshape(F)[None, :]),
        kw1=pack_w(kW1), kw2=pack_w(kW2),
        vw1=pack_w(vW1), vw2=pack_w(vW2),
        woqk=pack_hw(woq), wovv=pack_hw(wov),
        bq=np.ascontiguousarray(
            (np.asarray(kbo, dtype=f32) @ Wq_).reshape(1, F)),
        bkfs=pack_b(bkf, BETA), bkfu=pack_b(bkf, 1.0),
        kb1s=pack_b(kb1, BETA), kb1u=pack_b(kb1, 1.0),
        kb2u=pack_b(kb2, 1.0),
    )
    return nsup, xts, mks, evs, weights, core_bounds


_CACHE = {}
LAST_RESULT = None


def kernel(x, E, num_batch, batch_seg, Wq, Wkf, bkf, Wvf, kW1, kb1, kW2, kb2,
           kWo, kbo, vW1, vW2, vWo, **_ignored):
    from concourse.bass_utils import run_bass_kernel_spmd

    nsup, xts, mks, evs, weights, core_bounds = _prep_host(
        x, E, batch_seg, Wq, Wkf, bkf, Wvf, kW1, kb1, kW2, kb2, kWo, kbo,
        vW1, vW2, vWo)

    if nsup not in _CACHE:
        _CACHE[nsup] = _build_program(nsup)
    nc = _CACHE[nsup]

    in_maps = [
        dict(weights, x=xts[c], mk=mks[c], ev=evs[c])
        for c in range(NCORES)
    ]
    res = run_bass_kernel_spmd(nc, in_maps, core_ids=list(range(NCORES)))
    global LAST_RESULT
    LAST_RESULT = res

    N = np.asarray(x).shape[0]
    out = np.empty((N, F), dtype=np.float32)
    for c in range(NCORES):
        n0, n1 = core_bounds[c], core_bounds[c + 1]
        out[n0:n1] = res.results[c]["out"][:, :n1 - n0].T.astype(np.float32)
    return out
